# revision 1
# baseline (speedup 1.0000x reference)
"""Trainium2 Bass kernel for a single RoBERTa encoder layer.

Problem: B=8, S=512, H=1024, 16 heads (d=64), FF=4096, fp32, eval mode.

Strategy: pure data-parallel over batch — one batch element per NeuronCore
(8 cores), full weights replicated. Inside a core everything flows in a
"feature-on-partitions, tokens-on-free" transposed layout so that every
projection / FFN matmul takes weights in natural layout as the stationary
operand and activations as the moving operand with N=512 — which at
float32r (FP22 read of fp32 bits) runs at full PE rate (1 cycle/row).

Per-core pipeline:
  X [512,1024] --PE transpose--> XT (8 x [128,512] f32r)
  QT/KT = W.T @ XT (+bias via ACT Identity)      [h' on partitions]
  V'' normal layout with per-head [V_h | ones64] interleaved -> ctx matmul
  per head h: scoresT[kpos,q] = KT_h.T(slice) @ QT_h ; exp via ACT
              ctx' [128,512] = [V_h|ones].T @ expT  (rows 64:128 = sumexp)
              rinv = reciprocal(rows 64:128); ctxTh = psum[0:64] * rinv
  pack head pairs via selection matmuls ([I|0],[0|I]) -> ctxT pairs
  OT = Wo.T @ ctxT ; zT = OT + bo + XT (residual)
  LN1 in transposed layout: ones-matmul stats + K=1 broadcast matmuls
  interT = gelu(Wi.T @ attn_outT + bi) via ACT Gelu  [f on partitions]
  out2T = Wo2.T @ interT ; z2T = out2T + bo2 + attn_outT
  PE-transpose z2T -> z2 normal ; LN2 via bn_stats ; contiguous DMA out.
"""
import numpy as np

import concourse.bass as bass
import concourse.mybir as mybir
import concourse.tile as tile
from concourse import bacc
from concourse import bass_utils
from concourse.masks import make_identity

F32 = mybir.dt.float32
F32R = mybir.dt.float32r
AF = mybir.ActivationFunctionType
ALU = mybir.AluOpType

B, S, H, NH, HD, FF = 8, 512, 1024, 16, 64, 4096
KT_H = H // 128   # 8 k tiles over hidden
FT = FF // 128    # 32 f tiles over intermediate
ST = S // 128     # 4 token tiles
EPS = 1e-5

_CACHE = {}
DEBUG = False


def _build():
    nc = bacc.Bacc("TRN2", target_bir_lowering=False, debug=False,
                   enable_asserts=True, num_devices=B)

    def din(name, shape, dt=F32R):
        return nc.dram_tensor(name, shape, dt, kind="ExternalInput").ap()

    x = din("x", [S, H], F32)
    maskt = din("maskt", [128, ST], F32)        # per-core mask, host-transposed
    wq, wk, wv = din("wq", [H, H]), din("wk", [H, H]), din("wv", [H, H])
    wo, wi, wo2 = din("wo", [H, H]), din("wi", [H, FF]), din("wo2", [FF, H])
    bv = din("bv", [H], F32)
    # host-packed per-partition vectors: bq bk bo bo2 g1 b1 g2 b2 (8 cols each)
    # then bi (32 cols) -> [128, 96]
    biast = din("biast", [128, 96], F32)
    ones_col = din("ones_col", [128, 1])        # f32r ones
    ones_row = din("ones_row", [1, 128])        # f32r ones
    ones64 = din("ones64", [128, 64])           # f32r ones block for V''
    sel_a = din("sel_a", [128, 128])            # [[I64|0];[0|0]]
    sel_b = din("sel_b", [128, 128])            # [[0|I64];[0|0]]
    zeros64 = din("zeros64", [128, 64])         # f32r zeros
    out = nc.dram_tensor("out", [S, H], F32, kind="ExternalOutput").ap()
    dbg = {}
    if DEBUG:
        for nm, shp in [("d_xt0", [128, 512]), ("d_qt0", [128, 512]),
                        ("d_kt0", [128, 512]), ("d_vv0", [128, 2048]),
                        ("d_exp00", [128, 512]), ("d_ctx0", [128, 512]),
                        ("d_zt0", [128, 512]), ("d_aot0", [128, 512]),
                        ("d_int0", [128, 512]), ("d_ctxraw0", [128, 512]),
                        ("d_rinv0", [64, 512])]:
            dbg[nm] = nc.dram_tensor(nm, shp, F32, kind="ExternalOutput").ap()

    from contextlib import ExitStack
    es = ExitStack()
    with tile.TileContext(nc) as tc, es:
        # ---- long-lived pools (manually scoped via ExitStack.close) ----
        # LEFT stack: cst > p_xt > {phase transients} ... then p_int > {ffn}
        # RIGHT stack: p_aot > p_zt > p_pair (closed in reverse phase order)
        cst = es.enter_context(tc.tile_pool(name="cst", bufs=1))
        es_xt = ExitStack()
        p_xt = es_xt.enter_context(tc.tile_pool(name="p_xt", bufs=8))
        es_qk = ExitStack()
        es_pair = ExitStack()
        es_zt = ExitStack()
        es_aot = ExitStack()
        es_int = ExitStack()

        # ---- identity first (gates phase-1 transposes) ----
        ident = cst.tile([128, 128], F32, tag="ident")
        make_identity(nc, ident)

        # ---- phase 1: X -> XT (PE transposes) ----
        xt = [p_xt.tile([128, S], F32R, tag="xt", name=f"xt{i}") for i in range(KT_H)]
        with tc.tile_pool(name="p_x", bufs=2) as p_x, \
             tc.tile_pool(name="ps_tp", bufs=3, space="PSUM") as ps_tp:
            for i in range(ST):
                x_sb = p_x.tile([128, H], F32, tag="x")
                nc.sync.dma_start(out=x_sb, in_=x[128 * i:128 * (i + 1), :])
                for j in range(KT_H):
                    pt = ps_tp.tile([128, 128], F32, tag="tp")
                    nc.tensor.transpose(pt, x_sb[:, 128 * j:128 * (j + 1)], ident)
                    nc.vector.tensor_copy(out=xt[j][:, 128 * i:128 * (i + 1)], in_=pt)

        if DEBUG:
            nc.sync.dma_start(out=dbg["d_xt0"], in_=xt[0].bitcast(F32))

        # ---- constants / biases (emitted after x so x wins DMA priority) ----
        t_biast = cst.tile([128, 96], F32, tag="t_biast")
        nc.sync.dma_start(out=t_biast, in_=biast)
        t_bq, t_bk = t_biast[:, 0:8], t_biast[:, 8:16]
        t_bo, t_bo2 = t_biast[:, 16:24], t_biast[:, 24:32]
        t_g1, t_b1 = t_biast[:, 32:40], t_biast[:, 40:48]
        t_g2, t_b2 = t_biast[:, 48:56], t_biast[:, 56:64]
        t_bi = t_biast[:, 64:96]
        t_mask = cst.tile([128, ST], F32, tag="t_mask")
        nc.sync.dma_start(out=t_mask, in_=maskt)
        t_ones_col = cst.tile([128, 1], F32R, tag="t_ones_col")
        nc.sync.dma_start(out=t_ones_col, in_=ones_col)
        t_ones_row = cst.tile([1, 128], F32R, tag="t_ones_row")
        nc.sync.dma_start(out=t_ones_row, in_=ones_row)
        t_ones64 = cst.tile([128, 64], F32R, tag="t_ones64")
        nc.sync.dma_start(out=t_ones64, in_=ones64)
        t_sel_a = cst.tile([128, 128], F32R, tag="t_sel_a")
        nc.sync.dma_start(out=t_sel_a, in_=sel_a)
        t_sel_b = cst.tile([128, 128], F32R, tag="t_sel_b")
        nc.sync.dma_start(out=t_sel_b, in_=sel_b)
        t_zeros64 = cst.tile([128, 64], F32R, tag="t_zeros64")
        nc.sync.dma_start(out=t_zeros64, in_=zeros64)
        t_eps1 = cst.tile([1, 1], F32, tag="t_eps1")
        nc.vector.memset(t_eps1, EPS)
        t_eps128 = cst.tile([128, 1], F32, tag="t_eps128")
        nc.vector.memset(t_eps128, EPS)

        # ---- phase 2: QKV projections ----
        p_aot = es_aot.enter_context(tc.tile_pool(name="p_aot", bufs=8, side="right"))
        p_zt = es_zt.enter_context(tc.tile_pool(name="p_zt", bufs=8, side="right"))
        p_pair = es_pair.enter_context(tc.tile_pool(name="p_pair", bufs=8, side="right"))
        p_qt = es_qk.enter_context(tc.tile_pool(name="p_qt", bufs=16))
        p_kt = es_qk.enter_context(tc.tile_pool(name="p_kt", bufs=8))
        p_v = es_qk.enter_context(tc.tile_pool(name="p_v", bufs=4))
        qta = [p_qt.tile([128, S], F32R, tag="qt", name=f"qta{i}") for i in range(KT_H)]
        qtb = [p_qt.tile([128, S], F32R, tag="qt", name=f"qtb{i}") for i in range(KT_H)]
        kt = [p_kt.tile([128, S], F32R, tag="kt", name=f"ktt{i}") for i in range(KT_H)]
        for m in range(KT_H):  # zero the unused halves once
            nc.vector.tensor_copy(
                out=qta[m][64:128, :].rearrange("p (o c) -> p o c", c=64),
                in_=t_zeros64[64:128, :].rearrange(
                    "p (o c) -> p o c", o=1).broadcast_to([64, 8, 64]))
            nc.vector.tensor_copy(
                out=qtb[m][0:64, :].rearrange("p (o c) -> p o c", c=64),
                in_=t_zeros64[0:64, :].rearrange(
                    "p (o c) -> p o c", o=1).broadcast_to([64, 8, 64]))
        # V'' tiles: per s-tile [128, 2048]: head h at cols 128h..128h+64 = V,
        # cols 128h+64..128h+128 = ones
        vv = [p_v.tile([128, 16 * 128], F32R, tag="vv", name=f"vv{i}") for i in range(ST)]
        for s in range(ST):
            nc.vector.tensor_copy(
                out=vv[s].rearrange("p (h c) -> p h c", c=128)[:, :, 64:128],
                in_=t_ones64.rearrange("p (o c) -> p o c", o=1).broadcast_to(
                    [128, 16, 64]))

        with tc.tile_pool(name="p_w", bufs=12) as p_w, \
             tc.tile_pool(name="p_bv", bufs=1) as p_bv, \
             tc.tile_pool(name="ps_qkv", bufs=3, space="PSUM") as ps_qkv:
            # Q and K: transposed outputs, weights stationary
            t_bv_rep = p_bv.tile([128, H], F32, tag="t_bv_rep")
            nc.sync.dma_start(
                out=t_bv_rep,
                in_=bv.rearrange("(o n) -> o n", o=1).partition_broadcast(128))
            for w_dram, bias_t, which in ((wq, t_bq, "q"), (wk, t_bk, "k")):
                wt = []
                for k in range(KT_H):
                    w_sb = p_w.tile([128, H], F32R, tag="w4k")
                    nc.sync.dma_start(out=w_sb, in_=w_dram[128 * k:128 * (k + 1), :])
                    wt.append(w_sb)
                for m in range(KT_H):
                    ps = ps_qkv.tile([128, S], F32, tag="pq")
                    for k in range(KT_H):
                        nc.tensor.matmul(ps, wt[k][:, 128 * m:128 * (m + 1)], xt[k],
                                         start=(k == 0), stop=(k == KT_H - 1))
                    if which == "q":
                        nc.scalar.activation(out=qta[m][0:64, :], in_=ps[0:64, :],
                                             func=AF.Identity,
                                             bias=bias_t[0:64, m:m + 1], scale=1.0)
                        nc.scalar.activation(out=qtb[m][64:128, :], in_=ps[64:128, :],
                                             func=AF.Identity,
                                             bias=bias_t[64:128, m:m + 1], scale=1.0)
                    else:
                        nc.scalar.activation(out=kt[m], in_=ps, func=AF.Identity,
                                             bias=bias_t[:, m:m + 1], scale=1.0)
            # V: normal layout, X stationary
            wvt = []
            for k in range(KT_H):
                w_sb = p_w.tile([128, H], F32R, tag="w4k")
                nc.sync.dma_start(out=w_sb, in_=wv[128 * k:128 * (k + 1), :])
                wvt.append(w_sb)
            for s in range(ST):
                for n in range(2):
                    ps = ps_qkv.tile([128, 512], F32, tag="pq")
                    for k in range(KT_H):
                        nc.tensor.matmul(
                            ps, xt[k][:, 128 * s:128 * (s + 1)],
                            wvt[k][:, 512 * n:512 * (n + 1)],
                            start=(k == 0), stop=(k == KT_H - 1))
                    nc.vector.tensor_tensor(
                        out=vv[s].rearrange("p (h c) -> p h c", c=128)[:, 8 * n:8 * n + 8, 0:64],
                        in0=ps.rearrange("p (h c) -> p h c", c=64),
                        in1=t_bv_rep[:, 512 * n:512 * (n + 1)].rearrange(
                            "p (h c) -> p h c", c=64),
                        op=ALU.add)

        if DEBUG:
            nc.sync.dma_start(out=dbg["d_qt0"], in_=qta[0].bitcast(F32))
            nc.sync.dma_start(out=dbg["d_kt0"], in_=kt[0].bitcast(F32))
            nc.sync.dma_start(out=dbg["d_vv0"], in_=vv[0].bitcast(F32))

        # ---- phase 3: attention per head; pack pairs ----
        pair = [p_pair.tile([128, S], F32R, tag="pair", name=f"pair{i}") for i in range(KT_H)]
        with tc.tile_pool(name="p_expt", bufs=10) as p_expt, \
             tc.tile_pool(name="p_ctxh", bufs=1) as p_ctxh, \
             tc.tile_pool(name="p_rinv", bufs=4) as p_rinv, \
             tc.tile_pool(name="ps_sc", bufs=4, space="PSUM") as ps_sc, \
             tc.tile_pool(name="ps_ctx", bufs=2, space="PSUM") as ps_ctx, \
             tc.tile_pool(name="ps_shift", bufs=2, space="PSUM") as ps_shift:
            ctxh_t = [p_ctxh.tile([128, S], F32R, tag=f"ctxh{i}", name=f"ctxh{i}")
                      for i in range(4)]
            for i in range(4):  # zero rows 64:128 once (shift matmul safety)
                nc.vector.tensor_copy(
                    out=ctxh_t[i][64:128, :].rearrange("p (o c) -> p o c", c=64),
                    in_=t_zeros64[64:128, :].rearrange(
                        "p (o c) -> p o c", o=1).broadcast_to([64, 8, 64]))
            for t in range(KT_H):  # pair index
                expts = {}
                for h in (2 * t, 2 * t + 1):
                    mt = h // 2
                    qthalf = qta[mt] if h % 2 == 0 else qtb[mt]
                    expt = []
                    for kt_i in range(ST):
                        ps_s = ps_sc.tile([128, S], F32, tag="sc")
                        nc.tensor.matmul(
                            ps_s,
                            kt[mt][:, 128 * kt_i:128 * (kt_i + 1)],
                            qthalf,
                            start=True, stop=True)
                        e = p_expt.tile([128, S], F32R, tag="expt")
                        nc.scalar.activation(out=e, in_=ps_s, func=AF.Exp,
                                             bias=t_mask[:, kt_i:kt_i + 1],
                                             scale=1.0 / np.sqrt(HD))
                        expt.append(e)
                        if DEBUG and h == 0 and kt_i == 0:
                            nc.sync.dma_start(out=dbg["d_exp00"], in_=e.bitcast(F32))
                    expts[h] = expt
                ctxh = {}
                for h in (2 * t, 2 * t + 1):
                    expt = expts[h]
                    ps_c = ps_ctx.tile([128, S], F32, tag="ctx")
                    for kt_i in range(ST):
                        nc.tensor.matmul(
                            ps_c, vv[kt_i][:, 128 * h:128 * (h + 1)], expt[kt_i],
                            start=(kt_i == 0), stop=(kt_i == ST - 1))
                    if DEBUG and h == 0:
                        craw = p_ctxh.tile([128, S], F32, tag="craw")
                        nc.vector.tensor_copy(out=craw, in_=ps_c)
                        nc.sync.dma_start(out=dbg["d_ctxraw0"], in_=craw)
                    sums_sb = p_rinv.tile([64, S], F32, tag="sums_sb")
                    nc.vector.tensor_copy(out=sums_sb, in_=ps_c[64:128, :])
                    rinv = p_rinv.tile([64, S], F32, tag="rinv")
                    nc.vector.reciprocal_approx_fast(out=rinv, in_=sums_sb)
                    if DEBUG and h == 0:
                        nc.sync.dma_start(out=dbg["d_rinv0"], in_=rinv)
                    ch = ctxh_t[h % 4]
                    nc.vector.tensor_tensor(out=ch[0:64, :], in0=ps_c[0:64, :],
                                            in1=rinv, op=ALU.mult)
                    ctxh[h] = ch
                ps_p = ps_shift.tile([128, S], F32, tag="shift")
                nc.tensor.matmul(ps_p, t_sel_a, ctxh[2 * t], start=True, stop=False)
                nc.tensor.matmul(ps_p, t_sel_b, ctxh[2 * t + 1], start=False, stop=True)
                nc.vector.tensor_copy(out=pair[t], in_=ps_p)
                if DEBUG and t == 0:
                    nc.sync.dma_start(out=dbg["d_ctx0"], in_=pair[0].bitcast(F32))
        es_qk.close()

        # ---- phase 4: attention output proj + residual (transposed) ----
        zt = [p_zt.tile([128, S], F32R, tag="zt", name=f"zt{i}") for i in range(KT_H)]
        with tc.tile_pool(name="p_w2", bufs=9) as p_w2, \
             tc.tile_pool(name="p_tsum", bufs=2) as p_tsum, \
             tc.tile_pool(name="ps_wo", bufs=3, space="PSUM") as ps_wo:
            wot = []
            for k in range(KT_H):
                w_sb = p_w2.tile([128, H], F32R, tag="wo4k")
                nc.sync.dma_start(out=w_sb, in_=wo[128 * k:128 * (k + 1), :])
                wot.append(w_sb)
            for m in range(KT_H):
                ps = ps_wo.tile([128, S], F32, tag="wo")
                for k in range(KT_H):
                    nc.tensor.matmul(ps, wot[k][:, 128 * m:128 * (m + 1)], pair[k],
                                     start=(k == 0), stop=(k == KT_H - 1))
                # zT = OT + bo + XT
                tsum = p_tsum.tile([128, S], F32, tag="tsum")
                nc.vector.tensor_tensor(out=tsum, in0=ps, in1=xt[m], op=ALU.add)
                nc.scalar.activation(out=zt[m], in_=tsum, func=AF.Identity,
                                     bias=t_bo[:, m:m + 1], scale=1.0)
        if DEBUG:
            nc.sync.dma_start(out=dbg["d_zt0"], in_=zt[0].bitcast(F32))
        es_pair.close()
        es_xt.close()

        # ---- phase 5: LN1 in transposed layout ----
        aot = [p_aot.tile([128, S], F32R, tag="aot", name=f"aot{i}") for i in range(KT_H)]
        with tc.tile_pool(name="p_sq", bufs=2) as p_sq, \
             tc.tile_pool(name="p_stat", bufs=1) as p_stat, \
             tc.tile_pool(name="ps_stat", bufs=2, space="PSUM") as ps_stat, \
             tc.tile_pool(name="ps_rep", bufs=2, space="PSUM") as ps_rep:
            ps_sum = ps_stat.tile([1, S], F32, tag="lnsum")
            ps_sumsq = ps_stat.tile([1, S], F32, tag="lnsum")
            for m in range(KT_H):
                sq = p_sq.tile([128, S], F32R, tag="sq")
                nc.vector.tensor_tensor(out=sq, in0=zt[m], in1=zt[m], op=ALU.mult)
                nc.tensor.matmul(ps_sum, t_ones_col, zt[m],
                                 start=(m == 0), stop=(m == KT_H - 1))
                nc.tensor.matmul(ps_sumsq, t_ones_col, sq,
                                 start=(m == 0), stop=(m == KT_H - 1))
            mu = p_stat.tile([1, S], F32R, tag="mu")
            nc.vector.tensor_scalar(out=mu, in0=ps_sum, scalar1=1.0 / H,
                                    scalar2=None, op0=ALU.mult)
            ex2 = p_stat.tile([1, S], F32, tag="ex2")
            nc.vector.tensor_scalar(out=ex2, in0=ps_sumsq, scalar1=1.0 / H,
                                    scalar2=None, op0=ALU.mult)
            mu2 = p_stat.tile([1, S], F32, tag="mu2")
            nc.vector.tensor_tensor(out=mu2, in0=mu, in1=mu, op=ALU.mult)
            var = p_stat.tile([1, S], F32, tag="var")
            nc.vector.tensor_tensor(out=var, in0=ex2, in1=mu2, op=ALU.subtract)
            sd = p_stat.tile([1, S], F32, tag="sd")
            nc.scalar.activation(out=sd, in_=var, func=AF.Sqrt, bias=t_eps1, scale=1.0)
            rstd_f = p_stat.tile([1, S], F32, tag="rstd_f")
            nc.vector.reciprocal_approx_fast(out=rstd_f, in_=sd)
            rstd = p_stat.tile([1, S], F32R, tag="rstd")
            nc.scalar.activation(out=rstd, in_=rstd_f, func=AF.Identity)
            ps_mu = ps_rep.tile([128, S], F32, tag="murep")
            nc.tensor.matmul(ps_mu, t_ones_row, mu, start=True, stop=True)
            ps_rstd = ps_rep.tile([128, S], F32, tag="murep")
            nc.tensor.matmul(ps_rstd, t_ones_row, rstd, start=True, stop=True)
            for m in range(KT_H):
                t1 = p_sq.tile([128, S], F32, tag="t1")
                nc.vector.tensor_tensor(out=t1, in0=zt[m], in1=ps_mu, op=ALU.subtract)
                t2 = p_sq.tile([128, S], F32, tag="t2")
                nc.vector.tensor_tensor(out=t2, in0=t1, in1=ps_rstd, op=ALU.mult)
                nc.scalar.activation(out=aot[m], in_=t2, func=AF.Identity,
                                     bias=t_b1[:, m:m + 1], scale=t_g1[:, m:m + 1])
        es_zt.close()

        if DEBUG:
            nc.sync.dma_start(out=dbg["d_aot0"], in_=aot[0].bitcast(F32))

        # ---- phase 6: FFN1 (gelu) ----
        p_int = es_int.enter_context(tc.tile_pool(name="p_int", bufs=32))
        intert = [p_int.tile([128, S], F32R, tag="intert", name=f"intert{i}") for i in range(FT)]
        with tc.tile_pool(name="p_wi", bufs=16) as p_wi, \
             tc.tile_pool(name="ps_f1", bufs=4, space="PSUM") as ps_f1:
            for fb in range(FT // 4):  # blocks of 4 f-tiles
                wi_chunks = []
                for k in range(KT_H):
                    c = p_wi.tile([128, 512], F32R, tag="wi")
                    nc.sync.dma_start(
                        out=c, in_=wi[128 * k:128 * (k + 1), 512 * fb:512 * (fb + 1)])
                    wi_chunks.append(c)
                for fi in range(4):
                    f = 4 * fb + fi
                    ps = ps_f1.tile([128, S], F32, tag="f1")
                    for k in range(KT_H):
                        nc.tensor.matmul(
                            ps, wi_chunks[k][:, 128 * fi:128 * (fi + 1)], aot[k],
                            start=(k == 0), stop=(k == KT_H - 1))
                    nc.scalar.activation(out=intert[f], in_=ps, func=AF.Gelu,
                                         bias=t_bi[:, f:f + 1], scale=1.0)

        if DEBUG:
            nc.sync.dma_start(out=dbg["d_int0"], in_=intert[0].bitcast(F32))

        # ---- phase 7: FFN2 + residual + LN2 (transpose back) + out ----
        with tc.tile_pool(name="p_wo2", bufs=6) as p_wo2, \
             tc.tile_pool(name="p_z2t", bufs=8) as p_z2t, \
             tc.tile_pool(name="p_tsum2", bufs=3) as p_tsum2, \
             tc.tile_pool(name="p_res", bufs=4) as p_res:
            with tc.tile_pool(name="ps_f2", bufs=8, space="PSUM") as ps_f2:
                ps_o2 = [ps_f2.tile([128, S], F32, tag="f2", name=f"ps_o2_{i}") for i in range(KT_H)]
                for f in range(FT):
                    w_sb = p_wo2.tile([128, H], F32R, tag="wo2")
                    nc.sync.dma_start(out=w_sb, in_=wo2[128 * f:128 * (f + 1), :])
                    for m in range(KT_H):
                        nc.tensor.matmul(ps_o2[m], w_sb[:, 128 * m:128 * (m + 1)],
                                         intert[f], start=(f == 0), stop=(f == FT - 1))
                    z2t = []
                for m in range(KT_H):
                    tsum = p_tsum2.tile([128, S], F32, tag="tsum2")
                    nc.vector.tensor_tensor(out=tsum, in0=ps_o2[m], in1=aot[m], op=ALU.add)
                    zz = p_z2t.tile([128, S], F32R, tag="z2t")
                    nc.scalar.activation(out=zz, in_=tsum, func=AF.Identity,
                                         bias=t_bo2[:, m:m + 1], scale=1.0)
                    z2t.append(zz)
            # LN2 in transposed layout, then transpose result + store
            with tc.tile_pool(name="p_sq2", bufs=2) as p_sq2, \
                 tc.tile_pool(name="p_stat2", bufs=1) as p_stat2, \
                 tc.tile_pool(name="p_y", bufs=8) as p_y, \
                 tc.tile_pool(name="ps_stat2", bufs=2, space="PSUM") as ps_stat2, \
                 tc.tile_pool(name="ps_rep2", bufs=2, space="PSUM") as ps_rep2, \
                 tc.tile_pool(name="ps_tp2", bufs=3, space="PSUM") as ps_tp2:
                ps2_sum = ps_stat2.tile([1, S], F32, tag="ln2sum")
                ps2_sumsq = ps_stat2.tile([1, S], F32, tag="ln2sum")
                for m in range(KT_H):
                    sq2 = p_sq2.tile([128, S], F32R, tag="sq2")
                    nc.vector.tensor_tensor(out=sq2, in0=z2t[m], in1=z2t[m],
                                            op=ALU.mult)
                    nc.tensor.matmul(ps2_sum, t_ones_col, z2t[m],
                                     start=(m == 0), stop=(m == KT_H - 1))
                    nc.tensor.matmul(ps2_sumsq, t_ones_col, sq2,
                                     start=(m == 0), stop=(m == KT_H - 1))
                mu2t = p_stat2.tile([1, S], F32R, tag="mu2t")
                nc.vector.tensor_scalar(out=mu2t, in0=ps2_sum, scalar1=1.0 / H,
                                        scalar2=None, op0=ALU.mult)
                ex2b = p_stat2.tile([1, S], F32, tag="ex2b")
                nc.vector.tensor_scalar(out=ex2b, in0=ps2_sumsq, scalar1=1.0 / H,
                                        scalar2=None, op0=ALU.mult)
                mu2sq = p_stat2.tile([1, S], F32, tag="mu2sq")
                nc.vector.tensor_tensor(out=mu2sq, in0=mu2t, in1=mu2t, op=ALU.mult)
                var2 = p_stat2.tile([1, S], F32, tag="var2")
                nc.vector.tensor_tensor(out=var2, in0=ex2b, in1=mu2sq,
                                        op=ALU.subtract)
                sd2 = p_stat2.tile([1, S], F32, tag="sd2")
                nc.scalar.activation(out=sd2, in_=var2, func=AF.Sqrt,
                                     bias=t_eps1, scale=1.0)
                rstd2f = p_stat2.tile([1, S], F32, tag="rstd2f")
                nc.vector.reciprocal_approx_fast(out=rstd2f, in_=sd2)
                rstd2 = p_stat2.tile([1, S], F32R, tag="rstd2")
                nc.scalar.activation(out=rstd2, in_=rstd2f, func=AF.Identity)
                ps_mu2 = ps_rep2.tile([128, S], F32, tag="mu2rep")
                nc.tensor.matmul(ps_mu2, t_ones_row, mu2t, start=True, stop=True)
                ps_rstd2 = ps_rep2.tile([128, S], F32, tag="mu2rep")
                nc.tensor.matmul(ps_rstd2, t_ones_row, rstd2, start=True, stop=True)
                stg = [p_res.tile([128, H], F32, tag="stg", name=f"stg{i}")
                       for i in range(ST)]
                for m in range(KT_H):
                    u1 = p_sq2.tile([128, S], F32, tag="u1")
                    nc.vector.tensor_tensor(out=u1, in0=z2t[m], in1=ps_mu2,
                                            op=ALU.subtract)
                    u2 = p_sq2.tile([128, S], F32, tag="u2")
                    nc.vector.tensor_tensor(out=u2, in0=u1, in1=ps_rstd2,
                                            op=ALU.mult)
                    y = p_y.tile([128, S], F32, tag="y", name=f"y{m}")
                    nc.scalar.activation(out=y, in_=u2, func=AF.Identity,
                                         bias=t_b2[:, m:m + 1],
                                         scale=t_g2[:, m:m + 1])
                    for s_i in range(ST):
                        pt = ps_tp2.tile([128, 128], F32, tag="tp2")
                        nc.tensor.transpose(
                            pt, y[:, 128 * s_i:128 * (s_i + 1)], ident)
                        nc.vector.tensor_copy(
                            out=stg[s_i][:, 128 * m:128 * (m + 1)], in_=pt)
                for s_i in range(ST):
                    nc.sync.dma_start(out=out[128 * s_i:128 * (s_i + 1), :],
                                      in_=stg[s_i])
        es_int.close()
        es_aot.close()

    nc.compile()
    return nc


def _get_nc():
    if "nc" not in _CACHE:
        _CACHE["nc"] = _build()
    return _CACHE["nc"]


def _perpart(v):
    # [n*128] -> [128, n] with vT[p, t] = v[t*128 + p]
    v = np.asarray(v, np.float32)
    return np.ascontiguousarray(v.reshape(-1, 128).T)


def _shared_inputs(inp):
    f = np.float32
    biast = np.concatenate(
        [_perpart(inp["bq"]), _perpart(inp["bk"]), _perpart(inp["bo"]),
         _perpart(inp["bo2"]), _perpart(inp["ln1_g"]), _perpart(inp["ln1_b"]),
         _perpart(inp["ln2_g"]), _perpart(inp["ln2_b"]), _perpart(inp["bi"])],
        axis=1)
    return {
        "wq": np.ascontiguousarray(inp["wq"], f),
        "wk": np.ascontiguousarray(inp["wk"], f),
        "wv": np.ascontiguousarray(inp["wv"], f),
        "wo": np.ascontiguousarray(inp["wo"], f),
        "wi": np.ascontiguousarray(inp["wi"], f),
        "wo2": np.ascontiguousarray(inp["wo2"], f),
        "bv": np.ascontiguousarray(inp["bv"], f),
        "biast": biast,
        "ones_col": np.ones((128, 1), f),
        "ones_row": np.ones((1, 128), f),
        "ones64": np.ones((128, 64), f),
        "sel_a": np.concatenate(
            [np.concatenate([np.eye(64, dtype=f), np.zeros((64, 64), f)], axis=1),
             np.zeros((64, 128), f)], axis=0),
        "sel_b": np.concatenate(
            [np.concatenate([np.zeros((64, 64), f), np.eye(64, dtype=f)], axis=1),
             np.zeros((64, 128), f)], axis=0),
        "zeros64": np.zeros((128, 64), f),
    }


def kernel(hidden_states, attention_mask, wq, bq, wk, bk, wv, bv,
           wo, bo, ln1_g, ln1_b, wi, bi, wo2, bo2, ln2_g, ln2_b):
    nc = _get_nc()
    f = np.float32
    shared = _shared_inputs({
        "wq": wq, "wk": wk, "wv": wv, "wo": wo, "wi": wi, "wo2": wo2,
        "bq": bq, "bk": bk, "bv": bv, "bo": bo, "bi": bi, "bo2": bo2,
        "ln1_g": ln1_g, "ln1_b": ln1_b, "ln2_g": ln2_g, "ln2_b": ln2_b,
    })
    hs = np.ascontiguousarray(hidden_states, f)
    am = np.ascontiguousarray(attention_mask, f).reshape(B, S)
    in_maps = [dict(shared, x=hs[b], maskt=_perpart(am[b])) for b in range(B)]
    res = bass_utils.run_bass_kernel_spmd(nc, in_maps, core_ids=list(range(B)),
                                          trace=False)
    return np.stack([res.results[b]["out"] for b in range(B)]).astype(np.float32)



# revision 3
# speedup vs baseline: 1.1245x; 1.1245x over previous
"""Trainium2 Bass kernel for a single RoBERTa encoder layer.

Problem: B=8, S=512, H=1024, 16 heads (d=64), FF=4096, fp32 in/out, eval.

Strategy: data-parallel over batch (one batch element per core, 8 cores).
Per core, activations flow in a transposed "feature-on-partitions" layout.
Matmul dtypes: fp8e4 (e4m3) with DoubleRow perf mode (2x PE throughput)
for the QKV projections, the probs@V context matmul and the attention
output projection; bf16 (1 cyc/row) for scores, FFN1 and FFN2. Empirically
(see fp8 experiment) this config lands at ~7e-3 relative error vs the 2e-2
gate; fp8 in the FFN would exceed the budget.

Scaling tricks:
  - weights pre-scaled x16 before fp8 cast (avoids e4m3 subnormals),
    un-scaled for free via the ACT bias/scale path off PSUM.
  - exp() output scaled by exp(-2.34) so probs fit e4m3 nicely; softmax
    normalization (ones-rows trick inside the V'' matrix) cancels it.
  - host pre-transposes X and pre-packs all DoubleRow operand layouts.

Layout per core:
  xt (bf16, [h=128 x 8, tok 512])   transposed input, residual 1
  xt8 (fp8 pairs)                   QKV moving / V stationary operand
  qt/kt [feat 128, tok 512] bf16 -> scoresT[kpos, q] via 64-row stationary
  e8 = fp8(exp(scores/8 + mask - 2.34)) pairs -> ctx via DoubleRow with
    V''=[16(V+bv) | ones] -> ctx rows 0:64, sumexp rows 64:128
  pair8 = fp8(16*ctx) packed head pairs -> wo DoubleRow -> +bo +xt -> LN1
  FFN1 bf16 (wi stationary) -> gelu -> interT
  FFN2 bf16 activation-stationary (interT tiles) -> [tok, col] + aot^T
    + bo2 residual -> LN2 in normal layout -> bf16 out (host casts fp32)
"""
import math

import numpy as np
import ml_dtypes

import concourse.bass as bass
import concourse.mybir as mybir
import concourse.tile as tile
from concourse import bacc
from concourse import bass_utils
from concourse.masks import make_identity

F32 = mybir.dt.float32
BF16 = mybir.dt.bfloat16
FP8 = mybir.dt.float8e4
AF = mybir.ActivationFunctionType
ALU = mybir.AluOpType
DR = mybir.MatmulPerfMode.DoubleRow

B, S, H, NH, HD, FF = 8, 512, 1024, 16, 64, 4096
KT = H // 128       # 8 hidden k-tiles
PR = KT // 2        # 4 k-tile pairs (fp8 DoubleRow)
ST = S // 128       # 4 token tiles
FT = FF // 128      # 32 ff tiles
EPS = 1e-5
WS = 16.0           # weight prescale before fp8 cast
EXP_SHIFT = math.log(64.0) - 6.5   # folded into exp bias; cancels in softmax

_CACHE = {}


def _build():
    nc = bacc.Bacc("TRN2", target_bir_lowering=False, debug=False,
                   enable_asserts=True, num_devices=B)

    def din(name, shape, dt):
        return nc.dram_tensor(name, shape, dt, kind="ExternalInput").ap()

    # per-core inputs (host pre-packed)
    xt8_d = din("xt8", [512, 1024], FP8)        # pair-packed fp8 X^T
    xtb_d = din("xtb", [H, S], BF16)            # X^T bf16
    maskb_d = din("maskb", [128, ST], F32)      # per-kpos mask + EXP_SHIFT
    # shared weights
    wq8_d = din("wq8", [512, 2048], FP8)
    wk8_d = din("wk8", [512, 2048], FP8)
    wv8_d = din("wv8", [512, 2048], FP8)
    wo8_d = din("wo8", [512, 2048], FP8)
    wi_d = din("wi", [H, FF], BF16)
    wo2_d = din("wo2", [FF, H], BF16)
    # biast: bq bk bo g1 b1 (8 cols each) + bi (32) = 72 cols fp32
    biast_d = din("biast", [128, 72], F32)
    bv16_d = din("bv16", [1, H], BF16)          # 16*bv
    bo2r_d = din("bo2r", [1, H], BF16)
    g2r_d = din("g2r", [1, H], BF16)
    b2r_d = din("b2r", [1, H], BF16)
    sel_d = din("sel", [64, 256], BF16)         # [I64|0] then [0|I64]
    onec_d = din("onec", [128, 1], BF16)
    oner_d = din("oner", [1, 128], BF16)
    out_d = nc.dram_tensor("out", [S, H], BF16, kind="ExternalOutput").ap()

    from contextlib import ExitStack
    es = ExitStack()
    with tile.TileContext(nc) as tc, es:
        # ---------- long-lived pools ----------
        cst = es.enter_context(tc.tile_pool(name="cst", bufs=1))
        es_xtb = ExitStack()
        p_xtb = es_xtb.enter_context(tc.tile_pool(name="p_xtb", bufs=8))
        p_wo8 = es_xtb.enter_context(tc.tile_pool(name="p_wo8", bufs=4))
        p_pair8 = es_xtb.enter_context(tc.tile_pool(name="p_pair8", bufs=4))
        es_qkv = ExitStack()
        p_xt8 = es_qkv.enter_context(tc.tile_pool(name="p_xt8", bufs=4))
        p_w8 = es_qkv.enter_context(tc.tile_pool(name="p_w8", bufs=8))
        p_vv8 = es_qkv.enter_context(tc.tile_pool(name="p_vv8", bufs=2))
        # right side: FFN pools (reserve order for LIFO closes)
        es_int = ExitStack()
        p_int = es_int.enter_context(tc.tile_pool(name="p_int", bufs=32, side="right"))
        es_aot = ExitStack()
        p_aot = es_aot.enter_context(tc.tile_pool(name="p_aot", bufs=8, side="right"))

        # ---------- DMA in (sync queue), consumption order ----------
        xt8 = [p_xt8.tile([128, 1024], FP8, tag="xt8", name=f"xt8_{p}")
               for p in range(PR)]
        for p in range(PR):
            nc.sync.dma_start(out=xt8[p], in_=xt8_d[128 * p:128 * (p + 1), :])
        wv8 = [p_w8.tile([128, 2048], FP8, tag="w8", name=f"wv8_{p}")
               for p in range(PR)]
        for p in range(PR):
            nc.sync.dma_start(out=wv8[p], in_=wv8_d[128 * p:128 * (p + 1), :])
        wq8 = [p_w8.tile([128, 2048], FP8, tag="w8", name=f"wq8_{p}")
               for p in range(PR)]
        wk8 = [p_w8.tile([128, 2048], FP8, tag="w8", name=f"wk8_{p}")
               for p in range(PR)]
        for p in range(PR):
            nc.sync.dma_start(out=wq8[p], in_=wq8_d[128 * p:128 * (p + 1), :])
            nc.sync.dma_start(out=wk8[p], in_=wk8_d[128 * p:128 * (p + 1), :])
        xtb = [p_xtb.tile([128, S], BF16, tag="xtb", name=f"xtb{j}")
               for j in range(KT)]
        for j in range(KT):
            nc.sync.dma_start(out=xtb[j], in_=xtb_d[128 * j:128 * (j + 1), :])

        t_biast = cst.tile([128, 72], F32, tag="t_biast")
        nc.sync.dma_start(out=t_biast, in_=biast_d)
        t_bq, t_bk = t_biast[:, 0:8], t_biast[:, 8:16]
        t_bo = t_biast[:, 16:24]
        t_g1, t_b1 = t_biast[:, 24:32], t_biast[:, 32:40]
        t_bi = t_biast[:, 40:72]
        t_mask = cst.tile([128, ST], F32, tag="t_mask")
        nc.sync.dma_start(out=t_mask, in_=maskb_d)
        t_sel = cst.tile([64, 256], BF16, tag="t_sel")
        nc.sync.dma_start(out=t_sel, in_=sel_d)
        t_sel_a, t_sel_b = t_sel[:, 0:128], t_sel[:, 128:256]
        t_onec = cst.tile([128, 1], BF16, tag="t_onec")
        nc.sync.dma_start(out=t_onec, in_=onec_d)
        t_oner = cst.tile([1, 128], BF16, tag="t_oner")
        nc.sync.dma_start(out=t_oner, in_=oner_d)
        t_bv16 = cst.tile([128, H], BF16, tag="t_bv16")
        nc.sync.dma_start(out=t_bv16, in_=bv16_d.partition_broadcast(128))
        wo8 = [p_wo8.tile([128, 2048], FP8, tag="wo8", name=f"wo8_{p}")
               for p in range(PR)]
        for p in range(PR):
            nc.sync.dma_start(out=wo8[p], in_=wo8_d[128 * p:128 * (p + 1), :])
        t_bo2r = cst.tile([128, H], BF16, tag="t_bo2r")
        nc.sync.dma_start(out=t_bo2r, in_=bo2r_d.partition_broadcast(128))
        t_g2r = cst.tile([128, H], BF16, tag="t_g2r")
        nc.sync.dma_start(out=t_g2r, in_=g2r_d.partition_broadcast(128))
        t_b2r = cst.tile([128, H], BF16, tag="t_b2r")
        nc.sync.dma_start(out=t_b2r, in_=b2r_d.partition_broadcast(128))

        ident = cst.tile([128, 128], BF16, tag="ident")
        make_identity(nc, ident)
        t_eps1 = cst.tile([1, 1], F32, tag="t_eps1")
        nc.vector.memset(t_eps1, EPS)
        t_eps128 = cst.tile([128, 1], F32, tag="t_eps128")
        nc.vector.memset(t_eps128, EPS)

        # V'' tiles: [128 kpos, 2(kt pair), 16 heads, 128 (64 V | 64 ones)]
        vv8 = [p_vv8.tile([128, 4096], FP8, tag="vv8", name=f"vv8_{sp}")
               for sp in range(2)]
        for sp in range(2):
            nc.gpsimd.memset(
                vv8[sp].rearrange("p (i h c) -> p i h c", i=2, c=128)[:, :, :, 64:128],
                1.0)

        r3 = lambda t: t.rearrange("p (i n) -> p i n", i=2)

        # ---------- V projection (fp8 DoubleRow, activation-stationary) ----------
        with tc.tile_pool(name="ps_v", bufs=2, space="PSUM") as ps_v:
            for s in range(ST):
                for n in range(2):
                    ps = ps_v.tile([128, 512], F32, tag="psv")
                    for p in range(PR):
                        nc.tensor.matmul(
                            ps, r3(xt8[p])[:, :, 128 * s:128 * (s + 1)],
                            r3(wv8[p])[:, :, 512 * n:512 * (n + 1)],
                            start=(p == 0), stop=(p == PR - 1), perf_mode=DR)
                    nc.vector.tensor_tensor(
                        out=vv8[s // 2].rearrange(
                            "p (i h c) -> p i h c", i=2, c=128)[:, s % 2, 8 * n:8 * n + 8, 0:64],
                        in0=ps.rearrange("p (h c) -> p h c", c=64),
                        in1=t_bv16[:, 512 * n:512 * (n + 1)].rearrange(
                            "p (h c) -> p h c", c=64),
                        op=ALU.add)

        # ---------- per-pair QKV + attention ----------
        pair8 = [p_pair8.tile([128, 1024], FP8, tag="pair8", name=f"pair8_{j}")
                 for j in range(PR)]
        with tc.tile_pool(name="p_qt", bufs=3) as p_qt, \
             tc.tile_pool(name="p_kt", bufs=3) as p_kt, \
             tc.tile_pool(name="p_e8", bufs=5) as p_e8, \
             tc.tile_pool(name="p_ch", bufs=4) as p_ch, \
             tc.tile_pool(name="p_sums", bufs=4) as p_sums, \
             tc.tile_pool(name="ps_qk", bufs=2, space="PSUM") as ps_qk, \
             tc.tile_pool(name="ps_sc", bufs=2, space="PSUM") as ps_sc, \
             tc.tile_pool(name="ps_ctx", bufs=2, space="PSUM") as ps_ctx, \
             tc.tile_pool(name="ps_pk", bufs=2, space="PSUM") as ps_pk:
            for t in range(KT):
                qt_t = p_qt.tile([128, S], BF16, tag="qt")
                ps_q = ps_qk.tile([128, S], F32, tag="psq")
                for p in range(PR):
                    nc.tensor.matmul(ps_q, r3(wq8[p])[:, :, 128 * t:128 * (t + 1)],
                                     r3(xt8[p]), start=(p == 0),
                                     stop=(p == PR - 1), perf_mode=DR)
                nc.scalar.activation(out=qt_t, in_=ps_q, func=AF.Identity,
                                     bias=t_bq[:, t:t + 1], scale=1.0 / WS)
                kt_t = p_kt.tile([128, S], BF16, tag="kt")
                ps_k = ps_qk.tile([128, S], F32, tag="psq")
                for p in range(PR):
                    nc.tensor.matmul(ps_k, r3(wk8[p])[:, :, 128 * t:128 * (t + 1)],
                                     r3(xt8[p]), start=(p == 0),
                                     stop=(p == PR - 1), perf_mode=DR)
                nc.scalar.activation(out=kt_t, in_=ps_k, func=AF.Identity,
                                     bias=t_bk[:, t:t + 1], scale=1.0 / WS)
                ctxh = {}
                for hh in range(2):
                    lo, hi = 64 * hh, 64 * hh + 64
                    h = 2 * t + hh
                    e8 = [p_e8.tile([128, 1024], FP8, tag="e8", name=f"e8_{_i}")
                          for _i in range(2)]
                    for kt_i in range(ST):
                        ps_s = ps_sc.tile([128, S], F32, tag="sc")
                        nc.tensor.matmul(
                            ps_s, kt_t[lo:hi, 128 * kt_i:128 * (kt_i + 1)],
                            qt_t[lo:hi, :], start=True, stop=True)
                        nc.scalar.activation(
                            out=e8[kt_i // 2][:, 512 * (kt_i % 2):512 * (kt_i % 2 + 1)],
                            in_=ps_s, func=AF.Exp,
                            bias=t_mask[:, kt_i:kt_i + 1], scale=1.0 / 8.0)
                    ps_c = ps_ctx.tile([128, S], F32, tag="ctx")
                    for sp in range(2):
                        nc.tensor.matmul(
                            ps_c,
                            r3(vv8[sp])[:, :, 128 * h:128 * (h + 1)],
                            r3(e8[sp]),
                            start=(sp == 0), stop=(sp == 1), perf_mode=DR)
                    sums = p_sums.tile([64, S], F32, tag="sums")
                    nc.vector.tensor_copy(out=sums, in_=ps_c[64:128, :])
                    rinv = p_sums.tile([64, S], F32, tag="sums")
                    nc.vector.reciprocal_approx_fast(out=rinv, in_=sums)
                    ch = p_ch.tile([64, S], BF16, tag="ch")
                    nc.vector.tensor_tensor(out=ch, in0=ps_c[0:64, :], in1=rinv,
                                            op=ALU.mult)
                    ctxh[hh] = ch
                ps_p = ps_pk.tile([128, S], F32, tag="pk")
                nc.tensor.matmul(ps_p, t_sel_a, ctxh[0], start=True, stop=False)
                nc.tensor.matmul(ps_p, t_sel_b, ctxh[1], start=False, stop=True)
                nc.vector.tensor_copy(
                    out=pair8[t // 2][:, 512 * (t % 2):512 * (t % 2 + 1)], in_=ps_p)
        es_qkv.close()

        # ---------- attention output proj + residual + LN1 ----------
        zt = []
        with tc.tile_pool(name="p_zt", bufs=8) as p_zt, \
             tc.tile_pool(name="p_ztmp", bufs=3) as p_ztmp, \
             tc.tile_pool(name="p_sq", bufs=3) as p_sq, \
             tc.tile_pool(name="p_stat", bufs=1) as p_stat, \
             tc.tile_pool(name="p_rep", bufs=1) as p_rep, \
             tc.tile_pool(name="ps_wo", bufs=2, space="PSUM") as ps_wo, \
             tc.tile_pool(name="ps_stat", bufs=2, space="PSUM") as ps_stat, \
             tc.tile_pool(name="ps_rep", bufs=2, space="PSUM") as ps_rep:
            ps_sum = ps_stat.tile([1, S], F32, tag="lnsum")
            ps_sumsq = ps_stat.tile([1, S], F32, tag="lnsum")
            for m in range(KT):
                ps = ps_wo.tile([128, S], F32, tag="wo")
                for p in range(PR):
                    nc.tensor.matmul(ps, r3(wo8[p])[:, :, 128 * m:128 * (m + 1)],
                                     r3(pair8[p]), start=(p == 0),
                                     stop=(p == PR - 1), perf_mode=DR)
                ztmp = p_ztmp.tile([128, S], BF16, tag="ztmp")
                nc.scalar.activation(out=ztmp, in_=ps, func=AF.Identity,
                                     bias=t_bo[:, m:m + 1], scale=1.0 / (WS * WS))
                z = p_zt.tile([128, S], BF16, tag="zt", name=f"zt{m}")
                nc.vector.tensor_tensor(out=z, in0=ztmp, in1=xtb[m], op=ALU.add)
                zt.append(z)
                sq = p_sq.tile([128, S], BF16, tag="sq")
                nc.vector.tensor_tensor(out=sq, in0=z, in1=z, op=ALU.mult)
                nc.tensor.matmul(ps_sum, t_onec, z, start=(m == 0),
                                 stop=(m == KT - 1))
                nc.tensor.matmul(ps_sumsq, t_onec, sq, start=(m == 0),
                                 stop=(m == KT - 1))
            mu_f = p_stat.tile([1, S], F32, tag="mu_f")
            nc.vector.tensor_scalar(out=mu_f, in0=ps_sum, scalar1=1.0 / H,
                                    scalar2=None, op0=ALU.mult)
            mu_bf = p_stat.tile([1, S], BF16, tag="mu_bf")
            nc.vector.tensor_copy(out=mu_bf, in_=mu_f)
            ex2 = p_stat.tile([1, S], F32, tag="ex2")
            nc.vector.tensor_scalar(out=ex2, in0=ps_sumsq, scalar1=1.0 / H,
                                    scalar2=None, op0=ALU.mult)
            mu2 = p_stat.tile([1, S], F32, tag="mu2")
            nc.vector.tensor_tensor(out=mu2, in0=mu_f, in1=mu_f, op=ALU.mult)
            var = p_stat.tile([1, S], F32, tag="var")
            nc.vector.tensor_tensor(out=var, in0=ex2, in1=mu2, op=ALU.subtract)
            sd = p_stat.tile([1, S], F32, tag="sd")
            nc.scalar.activation(out=sd, in_=var, func=AF.Sqrt, bias=t_eps1,
                                 scale=1.0)
            rstd_f = p_stat.tile([1, S], F32, tag="rstd_f")
            nc.vector.reciprocal_approx_fast(out=rstd_f, in_=sd)
            rstd_bf = p_stat.tile([1, S], BF16, tag="rstd_bf")
            nc.vector.tensor_copy(out=rstd_bf, in_=rstd_f)
            ps_mu = ps_rep.tile([128, S], F32, tag="murep")
            nc.tensor.matmul(ps_mu, t_oner, mu_bf, start=True, stop=True)
            ps_rstd = ps_rep.tile([128, S], F32, tag="murep")
            nc.tensor.matmul(ps_rstd, t_oner, rstd_bf, start=True, stop=True)
            murep = p_rep.tile([128, S], BF16, tag="murep_sb")
            nc.vector.tensor_copy(out=murep, in_=ps_mu)
            rstdrep = p_rep.tile([128, S], BF16, tag="rstdrep_sb")
            nc.vector.tensor_copy(out=rstdrep, in_=ps_rstd)
            aot = [p_aot.tile([128, S], BF16, tag="aot", name=f"aot{m}")
                   for m in range(KT)]
            for m in range(KT):
                t1 = p_sq.tile([128, S], BF16, tag="t1")
                nc.vector.tensor_tensor(out=t1, in0=zt[m], in1=murep,
                                        op=ALU.subtract)
                t2 = p_sq.tile([128, S], BF16, tag="t2")
                nc.vector.tensor_tensor(out=t2, in0=t1, in1=rstdrep, op=ALU.mult)
                nc.vector.tensor_scalar(out=aot[m], in0=t2,
                                        scalar1=t_g1[:, m:m + 1],
                                        scalar2=t_b1[:, m:m + 1],
                                        op0=ALU.mult, op1=ALU.add)
        es_xtb.close()

        # ---------- aot transposes (normal-layout residual for FFN2) ----------
        p_aotbo = es.enter_context(tc.tile_pool(name="p_aotbo", bufs=4))
        aotbo = [p_aotbo.tile([128, H], BF16, tag="aotbo", name=f"aotbo{s}")
                 for s in range(ST)]
        with tc.tile_pool(name="ps_tp", bufs=3, space="PSUM") as ps_tp:
            for m in range(KT):
                for s in range(ST):
                    pt = ps_tp.tile([128, 128], BF16, tag="tp")
                    nc.tensor.transpose(pt, aot[m][:, 128 * s:128 * (s + 1)], ident)
                    nc.vector.tensor_tensor(
                        out=aotbo[s][:, 128 * m:128 * (m + 1)], in0=pt,
                        in1=t_bo2r[:, 128 * m:128 * (m + 1)], op=ALU.add)

        # ---------- FFN1 (bf16, wi stationary) ----------
        intert = [p_int.tile([128, S], BF16, tag="intert", name=f"int{f}")
                  for f in range(FT)]
        with tc.tile_pool(name="p_wi", bufs=16) as p_wi, \
             tc.tile_pool(name="ps_f1", bufs=3, space="PSUM") as ps_f1:
            for fb in range(FT // 4):
                wic = []
                for k in range(KT):
                    c = p_wi.tile([128, 512], BF16, tag="wi")
                    nc.sync.dma_start(
                        out=c, in_=wi_d[128 * k:128 * (k + 1),
                                        512 * fb:512 * (fb + 1)])
                    wic.append(c)
                for fi in range(4):
                    f = 4 * fb + fi
                    ps = ps_f1.tile([128, S], F32, tag="f1")
                    for k in range(KT):
                        nc.tensor.matmul(ps, wic[k][:, 128 * fi:128 * (fi + 1)],
                                         aot[k], start=(k == 0),
                                         stop=(k == KT - 1))
                    nc.scalar.activation(out=intert[f], in_=ps, func=AF.Gelu,
                                         bias=t_bi[:, f:f + 1], scale=1.0)
        es_aot.close()

        # ---------- FFN2 (bf16, activation-stationary) + residual + LN2 ----------
        with tc.tile_pool(name="p_wo2", bufs=6) as p_wo2, \
             tc.tile_pool(name="p_z2", bufs=4) as p_z2, \
             tc.tile_pool(name="p_sq2", bufs=2) as p_sq2, \
             tc.tile_pool(name="p_st2", bufs=4) as p_st2, \
             tc.tile_pool(name="p_y", bufs=3) as p_y, \
             tc.tile_pool(name="ps_f2", bufs=8, space="PSUM") as ps_f2:
            ps_o = [ps_f2.tile([128, 512], F32, tag="f2", name=f"ps_o{i}")
                    for i in range(8)]
            for f in range(FT):
                w2 = p_wo2.tile([128, H], BF16, tag="wo2")
                nc.gpsimd.dma_start(out=w2, in_=wo2_d[128 * f:128 * (f + 1), :])
                for s in range(ST):
                    lhsT = intert[f][:, 128 * s:128 * (s + 1)]
                    for hh in range(2):
                        nc.tensor.matmul(ps_o[2 * s + hh], lhsT,
                                         w2[:, 512 * hh:512 * (hh + 1)],
                                         start=(f == 0), stop=(f == FT - 1))
            for s in range(ST):
                z2 = p_z2.tile([128, H], BF16, tag="z2")
                for hh in range(2):
                    nc.vector.tensor_tensor(
                        out=z2[:, 512 * hh:512 * (hh + 1)], in0=ps_o[2 * s + hh],
                        in1=aotbo[s][:, 512 * hh:512 * (hh + 1)], op=ALU.add)
                s1 = p_st2.tile([128, 1], F32, tag="s1")
                nc.vector.tensor_reduce(out=s1, in_=z2, axis=mybir.AxisListType.X,
                                        op=ALU.add)
                sq2 = p_sq2.tile([128, H], BF16, tag="sq2")
                nc.vector.tensor_tensor(out=sq2, in0=z2, in1=z2, op=ALU.mult)
                s2 = p_st2.tile([128, 1], F32, tag="s2")
                nc.vector.tensor_reduce(out=s2, in_=sq2, axis=mybir.AxisListType.X,
                                        op=ALU.add)
                mu = p_st2.tile([128, 1], F32, tag="mu")
                nc.vector.tensor_scalar(out=mu, in0=s1, scalar1=1.0 / H,
                                        scalar2=None, op0=ALU.mult)
                ex2b = p_st2.tile([128, 1], F32, tag="ex2b")
                nc.vector.tensor_scalar(out=ex2b, in0=s2, scalar1=1.0 / H,
                                        scalar2=None, op0=ALU.mult)
                mu2b = p_st2.tile([128, 1], F32, tag="mu2b")
                nc.vector.tensor_tensor(out=mu2b, in0=mu, in1=mu, op=ALU.mult)
                varb = p_st2.tile([128, 1], F32, tag="varb")
                nc.vector.tensor_tensor(out=varb, in0=ex2b, in1=mu2b,
                                        op=ALU.subtract)
                sdb = p_st2.tile([128, 1], F32, tag="sdb")
                nc.scalar.activation(out=sdb, in_=varb, func=AF.Sqrt,
                                     bias=t_eps128, scale=1.0)
                rstdb = p_st2.tile([128, 1], F32, tag="rstdb")
                nc.vector.reciprocal(out=rstdb, in_=sdb)
                nmu = p_st2.tile([128, 1], F32, tag="nmu")
                nc.vector.tensor_tensor(out=nmu, in0=mu, in1=rstdb, op=ALU.mult)
                nb = p_st2.tile([128, 1], F32, tag="nb")
                nc.vector.tensor_scalar(out=nb, in0=nmu, scalar1=-1.0,
                                        scalar2=None, op0=ALU.mult)
                y1 = p_y.tile([128, H], BF16, tag="y1")
                nc.scalar.activation(out=y1, in_=z2, func=AF.Identity,
                                     bias=nb, scale=rstdb)
                y2 = p_y.tile([128, H], BF16, tag="y2")
                nc.vector.tensor_tensor(out=y2, in0=y1, in1=t_g2r, op=ALU.mult)
                y3 = p_y.tile([128, H], BF16, tag="y3")
                nc.vector.tensor_tensor(out=y3, in0=y2, in1=t_b2r, op=ALU.add)
                nc.sync.dma_start(out=out_d[128 * s:128 * (s + 1), :], in_=y3)
        es_int.close()

    nc.compile()
    return nc


def _get_nc():
    if "nc" not in _CACHE:
        _CACHE["nc"] = _build()
    return _CACHE["nc"]


def _perpart(v):
    # [n*128] -> [128, n] with vT[p, t] = v[t*128 + p]
    v = np.asarray(v, np.float32)
    return np.ascontiguousarray(v.reshape(-1, 128).T)


def _pack_pairs(w, scale, dtype):
    # w [K, N] -> [K/2, 2N] fp8: row (128p + k) holds [i in {0,1}, n] with
    # value w[(2p+i)*128 + k, n] * scale
    w = np.asarray(w, np.float32) * scale
    K, N = w.shape
    t = w.reshape(K // 256, 2, 128, N).transpose(0, 2, 1, 3)
    return np.ascontiguousarray(t.reshape(K // 2, 2 * N)).astype(dtype)


def _shared_inputs(inp):
    f8 = ml_dtypes.float8_e4m3
    bf = ml_dtypes.bfloat16
    f = np.float32
    biast = np.concatenate(
        [_perpart(inp["bq"]), _perpart(inp["bk"]), _perpart(inp["bo"]),
         _perpart(inp["ln1_g"]), _perpart(inp["ln1_b"]), _perpart(inp["bi"])],
        axis=1).astype(f)
    sel = np.zeros((64, 256), f)
    sel[:, 0:64] = np.eye(64)
    sel[:, 192:256] = np.eye(64)
    return {
        "wq8": _pack_pairs(inp["wq"], WS, f8),
        "wk8": _pack_pairs(inp["wk"], WS, f8),
        "wv8": _pack_pairs(inp["wv"], WS, f8),
        "wo8": _pack_pairs(inp["wo"], WS, f8),
        "wi": np.ascontiguousarray(np.asarray(inp["wi"], f)).astype(bf),
        "wo2": np.ascontiguousarray(np.asarray(inp["wo2"], f)).astype(bf),
        "biast": biast,
        "bv16": (np.asarray(inp["bv"], f) * WS).reshape(1, H).astype(bf),
        "bo2r": np.asarray(inp["bo2"], f).reshape(1, H).astype(bf),
        "g2r": np.asarray(inp["ln2_g"], f).reshape(1, H).astype(bf),
        "b2r": np.asarray(inp["ln2_b"], f).reshape(1, H).astype(bf),
        "sel": sel.astype(bf),
        "onec": np.ones((128, 1), f).astype(bf),
        "oner": np.ones((1, 128), f).astype(bf),
    }


def _core_inputs(shared, hs, am, b):
    f8 = ml_dtypes.float8_e4m3
    bf = ml_dtypes.bfloat16
    xT = np.ascontiguousarray(hs[b].T)  # [H, S] fp32
    return dict(
        shared,
        xt8=_pack_pairs(xT, 1.0, f8),
        xtb=xT.astype(bf),
        maskb=(_perpart(am[b]) + EXP_SHIFT).astype(np.float32),
    )


def kernel(hidden_states, attention_mask, wq, bq, wk, bk, wv, bv,
           wo, bo, ln1_g, ln1_b, wi, bi, wo2, bo2, ln2_g, ln2_b):
    nc = _get_nc()
    f = np.float32
    shared = _shared_inputs({
        "wq": wq, "wk": wk, "wv": wv, "wo": wo, "wi": wi, "wo2": wo2,
        "bq": bq, "bk": bk, "bv": bv, "bo": bo, "bi": bi, "bo2": bo2,
        "ln1_g": ln1_g, "ln1_b": ln1_b, "ln2_g": ln2_g, "ln2_b": ln2_b,
    })
    hs = np.ascontiguousarray(hidden_states, f)
    am = np.ascontiguousarray(attention_mask, f).reshape(B, S)
    in_maps = [_core_inputs(shared, hs, am, b) for b in range(B)]
    res = bass_utils.run_bass_kernel_spmd(nc, in_maps, core_ids=list(range(B)),
                                          trace=False)
    return np.stack([res.results[b]["out"].astype(f) for b in range(B)])


# revision 7
# speedup vs baseline: 1.2122x; 1.0780x over previous
"""Trainium2 Bass kernel for a single RoBERTa encoder layer.

Problem: B=8, S=512, H=1024, 16 heads (d=64), FF=4096, fp32 in/out, eval.

Strategy: data-parallel over batch (one batch element per core, 8 cores).
Per core, activations flow in a transposed "feature-on-partitions" layout.
Matmul dtypes: fp8e4 (e4m3) with DoubleRow perf mode (2x PE throughput)
for the QKV projections, the probs@V context matmul and the attention
output projection; bf16 (1 cyc/row) for scores, FFN1 and FFN2. Empirically
(see fp8 experiment) this config lands at ~7e-3 relative error vs the 2e-2
gate; fp8 in the FFN would exceed the budget.

Scaling tricks:
  - weights pre-scaled x16 before fp8 cast (avoids e4m3 subnormals),
    un-scaled for free via the ACT bias/scale path off PSUM.
  - exp() output scaled by exp(-2.34) so probs fit e4m3 nicely; softmax
    normalization (ones-rows trick inside the V'' matrix) cancels it.
  - host pre-transposes X and pre-packs all DoubleRow operand layouts.

Layout per core:
  xt (bf16, [h=128 x 8, tok 512])   transposed input, residual 1
  xt8 (fp8 pairs)                   QKV moving / V stationary operand
  qt/kt [feat 128, tok 512] bf16 -> scoresT[kpos, q] via 64-row stationary
  e8 = fp8(exp(scores/8 + mask - 2.34)) pairs -> ctx via DoubleRow with
    V''=[16(V+bv) | ones] -> ctx rows 0:64, sumexp rows 64:128
  pair8 = fp8(16*ctx) packed head pairs -> wo DoubleRow -> +bo +xt -> LN1
  FFN1 bf16 (wi stationary) -> gelu -> interT
  FFN2 bf16 activation-stationary (interT tiles) -> [tok, col] + aot^T
    + bo2 residual -> LN2 in normal layout -> bf16 out (host casts fp32)
"""
import math

import numpy as np
import ml_dtypes

import concourse.bass as bass
import concourse.mybir as mybir
import concourse.tile as tile
from concourse import bacc
from concourse import bass_utils
from concourse.masks import make_identity

F32 = mybir.dt.float32
BF16 = mybir.dt.bfloat16
FP8 = mybir.dt.float8e4
AF = mybir.ActivationFunctionType
ALU = mybir.AluOpType
DR = mybir.MatmulPerfMode.DoubleRow

B, S, H, NH, HD, FF = 8, 512, 1024, 16, 64, 4096
KT = H // 128       # 8 hidden k-tiles
PR = KT // 2        # 4 k-tile pairs (fp8 DoubleRow)
ST = S // 128       # 4 token tiles
FT = FF // 128      # 32 ff tiles
EPS = 1e-5
WS = 16.0           # weight prescale before fp8 cast
EXP_SHIFT = math.log(64.0) - 6.5   # folded into exp bias; cancels in softmax

_CACHE = {}


def _build():
    nc = bacc.Bacc("TRN2", target_bir_lowering=False, debug=False,
                   enable_asserts=True, num_devices=B)

    def din(name, shape, dt):
        return nc.dram_tensor(name, shape, dt, kind="ExternalInput").ap()

    # per-core inputs (host pre-packed)
    xt8_d = din("xt8", [512, 1024], FP8)        # pair-packed fp8 X^T
    xtb_d = din("xtb", [H, S], BF16)            # X^T bf16
    maskb_d = din("maskb", [128, ST], F32)      # per-kpos mask + EXP_SHIFT
    # shared weights
    wq8_d = din("wq8", [512, 2048], FP8)
    wk8_d = din("wk8", [512, 2048], FP8)
    wv8_d = din("wv8", [512, 2048], FP8)
    wo8_d = din("wo8", [512, 2048], FP8)
    wi_d = din("wi", [H, FF], BF16)
    wo2_d = din("wo2", [FF, H], BF16)
    # biast: bq bk bo g1 b1 (8 cols each) + bi (32) = 72 cols fp32
    biast_d = din("biast", [128, 72], F32)
    bv16_d = din("bv16", [1, H], BF16)          # 16*bv
    bo2r_d = din("bo2r", [1, H], BF16)
    g2r_d = din("g2r", [1, H], BF16)
    b2r_d = din("b2r", [1, H], BF16)
    sel_d = din("sel", [64, 256], BF16)         # [I64|0] then [0|I64]
    onec_d = din("onec", [128, 1], BF16)
    oner_d = din("oner", [1, 128], BF16)
    out_d = nc.dram_tensor("out", [S, H], BF16, kind="ExternalOutput").ap()

    from contextlib import ExitStack
    es = ExitStack()
    with tile.TileContext(nc) as tc, es:
        # ---------- long-lived pools ----------
        cst = es.enter_context(tc.tile_pool(name="cst", bufs=1))
        es_xtb = ExitStack()
        p_xtb = es_xtb.enter_context(tc.tile_pool(name="p_xtb", bufs=8))
        p_wo8 = es_xtb.enter_context(tc.tile_pool(name="p_wo8", bufs=4))
        p_pair8 = es_xtb.enter_context(tc.tile_pool(name="p_pair8", bufs=4))
        es_qkv = ExitStack()
        p_xt8 = es_qkv.enter_context(tc.tile_pool(name="p_xt8", bufs=4))
        p_w8 = es_qkv.enter_context(tc.tile_pool(name="p_w8", bufs=8))
        p_vv8 = es_qkv.enter_context(tc.tile_pool(name="p_vv8", bufs=2))
        # right side: FFN pools (reserve order for LIFO closes)
        es_int = ExitStack()
        p_int = es_int.enter_context(tc.tile_pool(name="p_int", bufs=32, side="right"))
        es_aot = ExitStack()
        p_aot = es_aot.enter_context(tc.tile_pool(name="p_aot", bufs=8, side="right"))

        # ---------- DMA in (sync queue), consumption order ----------
        xt8 = [p_xt8.tile([128, 1024], FP8, tag="xt8", name=f"xt8_{p}")
               for p in range(PR)]
        for p in range(PR):
            nc.sync.dma_start(out=xt8[p], in_=xt8_d[128 * p:128 * (p + 1), :])
        wv8 = [p_w8.tile([128, 2048], FP8, tag="w8", name=f"wv8_{p}")
               for p in range(PR)]
        for p in range(PR):
            nc.sync.dma_start(out=wv8[p], in_=wv8_d[128 * p:128 * (p + 1), :])
        wq8 = [p_w8.tile([128, 2048], FP8, tag="w8", name=f"wq8_{p}")
               for p in range(PR)]
        wk8 = [p_w8.tile([128, 2048], FP8, tag="w8", name=f"wk8_{p}")
               for p in range(PR)]
        for p in range(PR):
            nc.sync.dma_start(out=wq8[p], in_=wq8_d[128 * p:128 * (p + 1), :])
            nc.sync.dma_start(out=wk8[p], in_=wk8_d[128 * p:128 * (p + 1), :])
        xtb = [p_xtb.tile([128, S], BF16, tag="xtb", name=f"xtb{j}")
               for j in range(KT)]
        for j in range(KT):
            nc.sync.dma_start(out=xtb[j], in_=xtb_d[128 * j:128 * (j + 1), :])

        t_biast = cst.tile([128, 72], F32, tag="t_biast")
        nc.sync.dma_start(out=t_biast, in_=biast_d)
        t_bq, t_bk = t_biast[:, 0:8], t_biast[:, 8:16]
        t_bo = t_biast[:, 16:24]
        t_g1, t_b1 = t_biast[:, 24:32], t_biast[:, 32:40]
        t_bi = t_biast[:, 40:72]
        t_mask = cst.tile([128, ST], F32, tag="t_mask")
        nc.sync.dma_start(out=t_mask, in_=maskb_d)
        t_sel = cst.tile([64, 256], BF16, tag="t_sel")
        nc.sync.dma_start(out=t_sel, in_=sel_d)
        t_sel_a, t_sel_b = t_sel[:, 0:128], t_sel[:, 128:256]
        t_onec = cst.tile([128, 1], BF16, tag="t_onec")
        nc.sync.dma_start(out=t_onec, in_=onec_d)
        t_oner = cst.tile([1, 128], BF16, tag="t_oner")
        nc.sync.dma_start(out=t_oner, in_=oner_d)
        t_bv16 = cst.tile([128, H], BF16, tag="t_bv16")
        nc.sync.dma_start(out=t_bv16, in_=bv16_d.partition_broadcast(128))
        wo8 = [p_wo8.tile([128, 2048], FP8, tag="wo8", name=f"wo8_{p}")
               for p in range(PR)]
        for p in range(PR):
            nc.sync.dma_start(out=wo8[p], in_=wo8_d[128 * p:128 * (p + 1), :])
        t_bo2r = cst.tile([128, H], BF16, tag="t_bo2r")
        nc.sync.dma_start(out=t_bo2r, in_=bo2r_d.partition_broadcast(128))
        t_g2r = cst.tile([128, H], BF16, tag="t_g2r")
        nc.sync.dma_start(out=t_g2r, in_=g2r_d.partition_broadcast(128))
        t_b2r = cst.tile([128, H], BF16, tag="t_b2r")
        nc.sync.dma_start(out=t_b2r, in_=b2r_d.partition_broadcast(128))

        ident = cst.tile([128, 128], BF16, tag="ident")
        make_identity(nc, ident)
        t_eps1 = cst.tile([1, 1], F32, tag="t_eps1")
        nc.vector.memset(t_eps1, EPS)
        t_eps128 = cst.tile([128, 1], F32, tag="t_eps128")
        nc.vector.memset(t_eps128, EPS)

        # V'' tiles: [128 kpos, 2(kt pair), 16 heads, 128 (64 V | 64 ones)]
        vv8 = [p_vv8.tile([128, 4096], FP8, tag="vv8", name=f"vv8_{sp}")
               for sp in range(2)]
        for sp in range(2):
            nc.gpsimd.memset(
                vv8[sp].rearrange("p (i h c) -> p i h c", i=2, c=128)[:, :, :, 64:128],
                1.0)

        r3 = lambda t: t.rearrange("p (i n) -> p i n", i=2)

        # ---------- V projection (fp8 DoubleRow, activation-stationary) ----------
        with tc.tile_pool(name="ps_v", bufs=2, space="PSUM") as ps_v:
            for s in range(ST):
                for n in range(2):
                    ps = ps_v.tile([128, 512], F32, tag="psv")
                    for p in range(PR):
                        nc.tensor.matmul(
                            ps, r3(xt8[p])[:, :, 128 * s:128 * (s + 1)],
                            r3(wv8[p])[:, :, 512 * n:512 * (n + 1)],
                            start=(p == 0), stop=(p == PR - 1), perf_mode=DR)
                    nc.vector.tensor_tensor(
                        out=vv8[s // 2].rearrange(
                            "p (i h c) -> p i h c", i=2, c=128)[:, s % 2, 8 * n:8 * n + 8, 0:64],
                        in0=ps.rearrange("p (h c) -> p h c", c=64),
                        in1=t_bv16[:, 512 * n:512 * (n + 1)].rearrange(
                            "p (h c) -> p h c", c=64),
                        op=ALU.add)

        # ---------- Q/K projections (dense PE phase, biases on DVE) ----------
        pair8 = [p_pair8.tile([128, 1024], FP8, tag="pair8", name=f"pair8_{j}")
                 for j in range(PR)]
        with tc.tile_pool(name="p_qt", bufs=8) as p_qt, \
             tc.tile_pool(name="p_kt", bufs=8) as p_kt:
            qt = [p_qt.tile([128, S], BF16, tag="qt", name=f"qt{t}")
                  for t in range(KT)]
            kt = [p_kt.tile([128, S], BF16, tag="kt", name=f"kt{t}")
                  for t in range(KT)]
            with tc.tile_pool(name="ps_qk", bufs=3, space="PSUM") as ps_qk:
                for t in range(KT):
                    ps_q = ps_qk.tile([128, S], F32, tag="psq")
                    for p in range(PR):
                        nc.tensor.matmul(
                            ps_q, r3(wq8[p])[:, :, 128 * t:128 * (t + 1)],
                            r3(xt8[p]), start=(p == 0), stop=(p == PR - 1),
                            perf_mode=DR)
                    nc.vector.tensor_scalar(out=qt[t], in0=ps_q,
                                            scalar1=1.0 / WS,
                                            scalar2=t_bq[:, t:t + 1],
                                            op0=ALU.mult, op1=ALU.add)
                    ps_k = ps_qk.tile([128, S], F32, tag="psq")
                    for p in range(PR):
                        nc.tensor.matmul(
                            ps_k, r3(wk8[p])[:, :, 128 * t:128 * (t + 1)],
                            r3(xt8[p]), start=(p == 0), stop=(p == PR - 1),
                            perf_mode=DR)
                    nc.vector.tensor_scalar(out=kt[t], in0=ps_k,
                                            scalar1=1.0 / WS,
                                            scalar2=t_bk[:, t:t + 1],
                                            op0=ALU.mult, op1=ALU.add)

            # ---------- attention (2-head-batched exp) ----------
            with tc.tile_pool(name="p_e8", bufs=4) as p_e8, \
                 tc.tile_pool(name="p_ch", bufs=4) as p_ch, \
                 tc.tile_pool(name="p_sums", bufs=4) as p_sums, \
                 tc.tile_pool(name="ps_sc", bufs=2, space="PSUM") as ps_sc, \
                 tc.tile_pool(name="ps_ctx", bufs=2, space="PSUM") as ps_ctx, \
                 tc.tile_pool(name="ps_pk", bufs=2, space="PSUM") as ps_pk:
                for t in range(KT):
                    # e8c: [kpos, i(kt pair), qA | qB]
                    e8c = [p_e8.tile([128, 2048], FP8, tag="e8", name=f"e8_{_i}")
                           for _i in range(2)]
                    for kt_i in range(ST):
                        psc = ps_sc.tile([128, 1024], F32, tag="sc")
                        for hh in range(2):
                            lo, hi = 64 * hh, 64 * hh + 64
                            nc.tensor.matmul(
                                psc[:, 512 * hh:512 * (hh + 1)],
                                kt[t][lo:hi, 128 * kt_i:128 * (kt_i + 1)],
                                qt[t][lo:hi, :], start=True, stop=True)
                        nc.scalar.activation(
                            out=e8c[kt_i // 2].rearrange(
                                "p (i n) -> p i n", i=2)[:, kt_i % 2, :],
                            in_=psc, func=AF.Exp,
                            bias=t_mask[:, kt_i:kt_i + 1], scale=1.0 / 8.0)
                    ctxh = {}
                    for hh in range(2):
                        h = 2 * t + hh
                        ps_c = ps_ctx.tile([128, S], F32, tag="ctx")
                        for sp in range(2):
                            nc.tensor.matmul(
                                ps_c,
                                r3(vv8[sp])[:, :, 128 * h:128 * (h + 1)],
                                e8c[sp].rearrange(
                                    "p (i g n) -> p i g n", i=2, n=512)[:, :, hh, :],
                                start=(sp == 0), stop=(sp == 1), perf_mode=DR)
                        sums = p_sums.tile([64, S], F32, tag="sums")
                        nc.vector.tensor_copy(out=sums, in_=ps_c[64:128, :])
                        rinv = p_sums.tile([64, S], F32, tag="sums")
                        nc.vector.reciprocal_approx_fast(out=rinv, in_=sums)
                        ch = p_ch.tile([64, S], BF16, tag="ch")
                        nc.vector.tensor_tensor(out=ch, in0=ps_c[0:64, :],
                                                in1=rinv, op=ALU.mult)
                        ctxh[hh] = ch
                    ps_p = ps_pk.tile([128, S], F32, tag="pk")
                    nc.tensor.matmul(ps_p, t_sel_a, ctxh[0], start=True, stop=False)
                    nc.tensor.matmul(ps_p, t_sel_b, ctxh[1], start=False, stop=True)
                    nc.vector.tensor_copy(
                        out=pair8[t // 2][:, 512 * (t % 2):512 * (t % 2 + 1)],
                        in_=ps_p)
        es_qkv.close()

        # ---------- attention output proj + residual + LN1 ----------
        zt = []
        with tc.tile_pool(name="p_zt", bufs=8) as p_zt, \
             tc.tile_pool(name="p_ztmp", bufs=3) as p_ztmp, \
             tc.tile_pool(name="p_sq", bufs=3) as p_sq, \
             tc.tile_pool(name="p_stat", bufs=1) as p_stat, \
             tc.tile_pool(name="p_rep", bufs=1) as p_rep, \
             tc.tile_pool(name="ps_wo", bufs=2, space="PSUM") as ps_wo, \
             tc.tile_pool(name="ps_stat", bufs=2, space="PSUM") as ps_stat, \
             tc.tile_pool(name="ps_rep", bufs=2, space="PSUM") as ps_rep:
            ps_sum = ps_stat.tile([1, S], F32, tag="lnsum")
            ps_sumsq = ps_stat.tile([1, S], F32, tag="lnsum")
            for m in range(KT):
                ps = ps_wo.tile([128, S], F32, tag="wo")
                for p in range(PR):
                    nc.tensor.matmul(ps, r3(wo8[p])[:, :, 128 * m:128 * (m + 1)],
                                     r3(pair8[p]), start=(p == 0),
                                     stop=(p == PR - 1), perf_mode=DR)
                ztmp = p_ztmp.tile([128, S], BF16, tag="ztmp")
                nc.scalar.activation(out=ztmp, in_=ps, func=AF.Identity,
                                     bias=t_bo[:, m:m + 1], scale=1.0 / (WS * WS))
                z = p_zt.tile([128, S], BF16, tag="zt", name=f"zt{m}")
                nc.vector.tensor_tensor(out=z, in0=ztmp, in1=xtb[m], op=ALU.add)
                zt.append(z)
                sq = p_sq.tile([128, S], BF16, tag="sq")
                nc.vector.tensor_tensor(out=sq, in0=z, in1=z, op=ALU.mult)
                nc.tensor.matmul(ps_sum, t_onec, z, start=(m == 0),
                                 stop=(m == KT - 1))
                nc.tensor.matmul(ps_sumsq, t_onec, sq, start=(m == 0),
                                 stop=(m == KT - 1))
            mu_f = p_stat.tile([1, S], F32, tag="mu_f")
            nc.vector.tensor_scalar(out=mu_f, in0=ps_sum, scalar1=1.0 / H,
                                    scalar2=None, op0=ALU.mult)
            mu_bf = p_stat.tile([1, S], BF16, tag="mu_bf")
            nc.vector.tensor_copy(out=mu_bf, in_=mu_f)
            ex2 = p_stat.tile([1, S], F32, tag="ex2")
            nc.vector.tensor_scalar(out=ex2, in0=ps_sumsq, scalar1=1.0 / H,
                                    scalar2=None, op0=ALU.mult)
            mu2 = p_stat.tile([1, S], F32, tag="mu2")
            nc.vector.tensor_tensor(out=mu2, in0=mu_f, in1=mu_f, op=ALU.mult)
            var = p_stat.tile([1, S], F32, tag="var")
            nc.vector.tensor_tensor(out=var, in0=ex2, in1=mu2, op=ALU.subtract)
            sd = p_stat.tile([1, S], F32, tag="sd")
            nc.scalar.activation(out=sd, in_=var, func=AF.Sqrt, bias=t_eps1,
                                 scale=1.0)
            rstd_f = p_stat.tile([1, S], F32, tag="rstd_f")
            nc.vector.reciprocal_approx_fast(out=rstd_f, in_=sd)
            rstd_bf = p_stat.tile([1, S], BF16, tag="rstd_bf")
            nc.vector.tensor_copy(out=rstd_bf, in_=rstd_f)
            ps_mu = ps_rep.tile([128, S], F32, tag="murep")
            nc.tensor.matmul(ps_mu, t_oner, mu_bf, start=True, stop=True)
            ps_rstd = ps_rep.tile([128, S], F32, tag="murep")
            nc.tensor.matmul(ps_rstd, t_oner, rstd_bf, start=True, stop=True)
            murep = p_rep.tile([128, S], BF16, tag="murep_sb")
            nc.vector.tensor_copy(out=murep, in_=ps_mu)
            rstdrep = p_rep.tile([128, S], BF16, tag="rstdrep_sb")
            nc.vector.tensor_copy(out=rstdrep, in_=ps_rstd)
            aot = [p_aot.tile([128, S], BF16, tag="aot", name=f"aot{m}")
                   for m in range(KT)]
            for m in range(KT):
                t1 = p_sq.tile([128, S], BF16, tag="t1")
                nc.vector.tensor_tensor(out=t1, in0=zt[m], in1=murep,
                                        op=ALU.subtract)
                t2 = p_sq.tile([128, S], BF16, tag="t2")
                nc.vector.tensor_tensor(out=t2, in0=t1, in1=rstdrep, op=ALU.mult)
                nc.vector.tensor_scalar(out=aot[m], in0=t2,
                                        scalar1=t_g1[:, m:m + 1],
                                        scalar2=t_b1[:, m:m + 1],
                                        op0=ALU.mult, op1=ALU.add)
        es_xtb.close()

        # ---------- aot transposes (normal-layout residual for FFN2) ----------
        p_aotbo = es.enter_context(tc.tile_pool(name="p_aotbo", bufs=4))
        aotbo = [p_aotbo.tile([128, H], BF16, tag="aotbo", name=f"aotbo{s}")
                 for s in range(ST)]
        with tc.tile_pool(name="ps_tp", bufs=3, space="PSUM") as ps_tp:
            for m in range(KT):
                for s in range(ST):
                    pt = ps_tp.tile([128, 128], BF16, tag="tp")
                    nc.tensor.transpose(pt, aot[m][:, 128 * s:128 * (s + 1)], ident)
                    nc.vector.tensor_tensor(
                        out=aotbo[s][:, 128 * m:128 * (m + 1)], in0=pt,
                        in1=t_bo2r[:, 128 * m:128 * (m + 1)], op=ALU.add)

        # ---------- FFN1 (bf16, wi stationary) ----------
        # wo2 fully resident for FFN2 (DMA'd during FFN1 on the gpsimd queue)
        p_wo2r = es.enter_context(tc.tile_pool(name="p_wo2r", bufs=32))
        wo2r = [p_wo2r.tile([128, H], BF16, tag="wo2r", name=f"wo2r{f}")
                for f in range(FT)]
        for f in range(FT):
            nc.gpsimd.dma_start(out=wo2r[f], in_=wo2_d[128 * f:128 * (f + 1), :])
        intert = [p_int.tile([128, S], BF16, tag="intert", name=f"int{f}")
                  for f in range(FT)]
        with tc.tile_pool(name="p_wi", bufs=16) as p_wi, \
             tc.tile_pool(name="ps_f1", bufs=3, space="PSUM") as ps_f1:
            for fb in range(FT // 4):
                wic = []
                for k in range(KT):
                    c = p_wi.tile([128, 512], BF16, tag="wi")
                    nc.sync.dma_start(
                        out=c, in_=wi_d[128 * k:128 * (k + 1),
                                        512 * fb:512 * (fb + 1)])
                    wic.append(c)
                for fi in range(4):
                    f = 4 * fb + fi
                    ps = ps_f1.tile([128, S], F32, tag="f1")
                    for k in range(KT):
                        nc.tensor.matmul(ps, wic[k][:, 128 * fi:128 * (fi + 1)],
                                         aot[k], start=(k == 0),
                                         stop=(k == KT - 1))
                    nc.scalar.activation(out=intert[f], in_=ps, func=AF.Gelu,
                                         bias=t_bi[:, f:f + 1], scale=1.0)
        es_aot.close()

        # ---------- FFN2 (bf16, activation-stationary, s-outer) + LN2 ----------
        with tc.tile_pool(name="p_z2", bufs=3) as p_z2, \
             tc.tile_pool(name="p_sq2", bufs=2) as p_sq2, \
             tc.tile_pool(name="p_st2", bufs=4) as p_st2, \
             tc.tile_pool(name="p_y", bufs=3) as p_y, \
             tc.tile_pool(name="ps_f2", bufs=4, space="PSUM") as ps_f2:
            for s in range(ST):
                ps_a = ps_f2.tile([128, 512], F32, tag="f2")
                ps_b = ps_f2.tile([128, 512], F32, tag="f2")
                for f in range(FT):
                    lhsT = intert[f][:, 128 * s:128 * (s + 1)]
                    nc.tensor.matmul(ps_a, lhsT, wo2r[f][:, 0:512],
                                     start=(f == 0), stop=(f == FT - 1))
                    nc.tensor.matmul(ps_b, lhsT, wo2r[f][:, 512:1024],
                                     start=(f == 0), stop=(f == FT - 1))
                z2 = p_z2.tile([128, H], BF16, tag="z2")
                for hh, pso in ((0, ps_a), (1, ps_b)):
                    nc.vector.tensor_tensor(
                        out=z2[:, 512 * hh:512 * (hh + 1)], in0=pso,
                        in1=aotbo[s][:, 512 * hh:512 * (hh + 1)], op=ALU.add)
                s1 = p_st2.tile([128, 1], F32, tag="s1")
                nc.vector.tensor_reduce(out=s1, in_=z2, axis=mybir.AxisListType.X,
                                        op=ALU.add)
                sq2 = p_sq2.tile([128, H], BF16, tag="sq2")
                nc.vector.tensor_tensor(out=sq2, in0=z2, in1=z2, op=ALU.mult)
                s2 = p_st2.tile([128, 1], F32, tag="s2")
                nc.vector.tensor_reduce(out=s2, in_=sq2, axis=mybir.AxisListType.X,
                                        op=ALU.add)
                mu = p_st2.tile([128, 1], F32, tag="mu")
                nc.vector.tensor_scalar(out=mu, in0=s1, scalar1=1.0 / H,
                                        scalar2=None, op0=ALU.mult)
                ex2b = p_st2.tile([128, 1], F32, tag="ex2b")
                nc.vector.tensor_scalar(out=ex2b, in0=s2, scalar1=1.0 / H,
                                        scalar2=None, op0=ALU.mult)
                mu2b = p_st2.tile([128, 1], F32, tag="mu2b")
                nc.vector.tensor_tensor(out=mu2b, in0=mu, in1=mu, op=ALU.mult)
                varb = p_st2.tile([128, 1], F32, tag="varb")
                nc.vector.tensor_tensor(out=varb, in0=ex2b, in1=mu2b,
                                        op=ALU.subtract)
                sdb = p_st2.tile([128, 1], F32, tag="sdb")
                nc.scalar.activation(out=sdb, in_=varb, func=AF.Sqrt,
                                     bias=t_eps128, scale=1.0)
                rstdb = p_st2.tile([128, 1], F32, tag="rstdb")
                nc.vector.reciprocal(out=rstdb, in_=sdb)
                nmu = p_st2.tile([128, 1], F32, tag="nmu")
                nc.vector.tensor_tensor(out=nmu, in0=mu, in1=rstdb, op=ALU.mult)
                nb = p_st2.tile([128, 1], F32, tag="nb")
                nc.vector.tensor_scalar(out=nb, in0=nmu, scalar1=-1.0,
                                        scalar2=None, op0=ALU.mult)
                y1 = p_y.tile([128, H], BF16, tag="y1")
                nc.scalar.activation(out=y1, in_=z2, func=AF.Identity,
                                     bias=nb, scale=rstdb)
                y2 = p_y.tile([128, H], BF16, tag="y2")
                nc.vector.tensor_tensor(out=y2, in0=y1, in1=t_g2r, op=ALU.mult)
                y3 = p_y.tile([128, H], BF16, tag="y3")
                nc.vector.tensor_tensor(out=y3, in0=y2, in1=t_b2r, op=ALU.add)
                nc.sync.dma_start(out=out_d[128 * s:128 * (s + 1), :], in_=y3)
        es_int.close()

    nc.compile()
    return nc


def _get_nc():
    if "nc" not in _CACHE:
        _CACHE["nc"] = _build()
    return _CACHE["nc"]


def _perpart(v):
    # [n*128] -> [128, n] with vT[p, t] = v[t*128 + p]
    v = np.asarray(v, np.float32)
    return np.ascontiguousarray(v.reshape(-1, 128).T)


def _pack_pairs(w, scale, dtype):
    # w [K, N] -> [K/2, 2N] fp8: row (128p + k) holds [i in {0,1}, n] with
    # value w[(2p+i)*128 + k, n] * scale
    w = np.asarray(w, np.float32) * scale
    K, N = w.shape
    t = w.reshape(K // 256, 2, 128, N).transpose(0, 2, 1, 3)
    return np.ascontiguousarray(t.reshape(K // 2, 2 * N)).astype(dtype)


def _shared_inputs(inp):
    f8 = ml_dtypes.float8_e4m3
    bf = ml_dtypes.bfloat16
    f = np.float32
    biast = np.concatenate(
        [_perpart(inp["bq"]), _perpart(inp["bk"]), _perpart(inp["bo"]),
         _perpart(inp["ln1_g"]), _perpart(inp["ln1_b"]), _perpart(inp["bi"])],
        axis=1).astype(f)
    sel = np.zeros((64, 256), f)
    sel[:, 0:64] = np.eye(64)
    sel[:, 192:256] = np.eye(64)
    return {
        "wq8": _pack_pairs(inp["wq"], WS, f8),
        "wk8": _pack_pairs(inp["wk"], WS, f8),
        "wv8": _pack_pairs(inp["wv"], WS, f8),
        "wo8": _pack_pairs(inp["wo"], WS, f8),
        "wi": np.ascontiguousarray(np.asarray(inp["wi"], f)).astype(bf),
        "wo2": np.ascontiguousarray(np.asarray(inp["wo2"], f)).astype(bf),
        "biast": biast,
        "bv16": (np.asarray(inp["bv"], f) * WS).reshape(1, H).astype(bf),
        "bo2r": np.asarray(inp["bo2"], f).reshape(1, H).astype(bf),
        "g2r": np.asarray(inp["ln2_g"], f).reshape(1, H).astype(bf),
        "b2r": np.asarray(inp["ln2_b"], f).reshape(1, H).astype(bf),
        "sel": sel.astype(bf),
        "onec": np.ones((128, 1), f).astype(bf),
        "oner": np.ones((1, 128), f).astype(bf),
    }


def _core_inputs(shared, hs, am, b):
    f8 = ml_dtypes.float8_e4m3
    bf = ml_dtypes.bfloat16
    xT = np.ascontiguousarray(hs[b].T)  # [H, S] fp32
    return dict(
        shared,
        xt8=_pack_pairs(xT, 1.0, f8),
        xtb=xT.astype(bf),
        maskb=(_perpart(am[b]) + EXP_SHIFT).astype(np.float32),
    )


def kernel(hidden_states, attention_mask, wq, bq, wk, bk, wv, bv,
           wo, bo, ln1_g, ln1_b, wi, bi, wo2, bo2, ln2_g, ln2_b):
    nc = _get_nc()
    f = np.float32
    shared = _shared_inputs({
        "wq": wq, "wk": wk, "wv": wv, "wo": wo, "wi": wi, "wo2": wo2,
        "bq": bq, "bk": bk, "bv": bv, "bo": bo, "bi": bi, "bo2": bo2,
        "ln1_g": ln1_g, "ln1_b": ln1_b, "ln2_g": ln2_g, "ln2_b": ln2_b,
    })
    hs = np.ascontiguousarray(hidden_states, f)
    am = np.ascontiguousarray(attention_mask, f).reshape(B, S)
    in_maps = [_core_inputs(shared, hs, am, b) for b in range(B)]
    res = bass_utils.run_bass_kernel_spmd(nc, in_maps, core_ids=list(range(B)),
                                          trace=False)
    return np.stack([res.results[b]["out"].astype(f) for b in range(B)])


# revision 14
# speedup vs baseline: 1.2882x; 1.0627x over previous
"""Trainium2 Bass kernel for a single RoBERTa encoder layer.

Problem: B=8, S=512, H=1024, 16 heads (d=64), FF=4096, fp32 in/out, eval.

Strategy: data-parallel over batch (one batch element per core, 8 cores).
Per core, activations flow in a transposed "feature-on-partitions" layout.
Matmul dtypes: fp8e4 (e4m3) with DoubleRow perf mode (2x PE throughput)
for the QKV projections, the probs@V context matmul and the attention
output projection; bf16 (1 cyc/row) for scores, FFN1 and FFN2. Empirically
(see fp8 experiment) this config lands at ~7e-3 relative error vs the 2e-2
gate; fp8 in the FFN would exceed the budget.

Scaling tricks:
  - weights pre-scaled x16 before fp8 cast (avoids e4m3 subnormals),
    un-scaled for free via the ACT bias/scale path off PSUM.
  - exp() output scaled by exp(-2.34) so probs fit e4m3 nicely; softmax
    normalization (ones-rows trick inside the V'' matrix) cancels it.
  - host pre-transposes X and pre-packs all DoubleRow operand layouts.

Layout per core:
  xt (bf16, [h=128 x 8, tok 512])   transposed input, residual 1
  xt8 (fp8 pairs)                   QKV moving / V stationary operand
  qt/kt [feat 128, tok 512] bf16 -> scoresT[kpos, q] via 64-row stationary
  e8 = fp8(exp(scores/8 + mask - 2.34)) pairs -> ctx via DoubleRow with
    V''=[16(V+bv) | ones] -> ctx rows 0:64, sumexp rows 64:128
  pair8 = fp8(16*ctx) packed head pairs -> wo DoubleRow -> +bo +xt -> LN1
  FFN1 bf16 (wi stationary) -> gelu -> interT
  FFN2 bf16 activation-stationary (interT tiles) -> [tok, col] + aot^T
    + bo2 residual -> LN2 in normal layout -> bf16 out (host casts fp32)
"""
import math

import numpy as np
import ml_dtypes

import concourse.bass as bass
import concourse.mybir as mybir
import concourse.tile as tile
from concourse import bacc
from concourse import bass_utils
from concourse.masks import make_identity

F32 = mybir.dt.float32
BF16 = mybir.dt.bfloat16
FP8 = mybir.dt.float8e4
AF = mybir.ActivationFunctionType
ALU = mybir.AluOpType
DR = mybir.MatmulPerfMode.DoubleRow

B, S, H, NH, HD, FF = 8, 512, 1024, 16, 64, 4096
KT = H // 128       # 8 hidden k-tiles
PR = KT // 2        # 4 k-tile pairs (fp8 DoubleRow)
ST = S // 128       # 4 token tiles
FT = FF // 128      # 32 ff tiles
EPS = 1e-5
WS = 16.0           # weight prescale before fp8 cast
EXP_SHIFT = math.log(64.0) - 6.5   # folded into exp bias; cancels in softmax

_CACHE = {}


def _build():
    nc = bacc.Bacc("TRN2", target_bir_lowering=False, debug=False,
                   enable_asserts=True, num_devices=B)

    def din(name, shape, dt):
        return nc.dram_tensor(name, shape, dt, kind="ExternalInput").ap()

    # per-core inputs (host pre-packed)
    xt8_d = din("xt8", [512, 1024], FP8)        # pair-packed fp8 X^T
    xtb_d = din("xtb", [H, S], BF16)            # X^T bf16
    maskb_d = din("maskb", [128, ST], F32)      # per-kpos mask + EXP_SHIFT
    # shared weights
    wq8_d = din("wq8", [512, 2048], FP8)
    wk8_d = din("wk8", [512, 2048], FP8)
    wv8_d = din("wv8", [512, 2048], FP8)
    wo8_d = din("wo8", [512, 2048], FP8)
    wi_d = din("wi", [H, FF], BF16)
    wo2_d = din("wo2", [FF, H], BF16)
    # biast: bq bk bo g1 b1 (8 cols each) + bi (32) = 72 cols fp32
    biast_d = din("biast", [128, 72], F32)
    bv16_d = din("bv16", [1, H], BF16)          # 16*bv
    bo2r_d = din("bo2r", [1, H], BF16)
    g2r_d = din("g2r", [1, H], BF16)
    b2r_d = din("b2r", [1, H], BF16)
    sel_d = din("sel", [64, 256], BF16)         # [I64|0] then [0|I64]
    onec_d = din("onec", [128, 1], BF16)
    oner_d = din("oner", [1, 128], BF16)
    out_d = nc.dram_tensor("out", [S, H], BF16, kind="ExternalOutput").ap()

    from contextlib import ExitStack
    es = ExitStack()
    with tile.TileContext(nc) as tc, es:
        # ---------- long-lived pools ----------
        cst = es.enter_context(tc.tile_pool(name="cst", bufs=1))
        es_xtb = ExitStack()
        p_xtb = es_xtb.enter_context(tc.tile_pool(name="p_xtb", bufs=1))
        p_wo8 = es_xtb.enter_context(tc.tile_pool(name="p_wo8", bufs=1))
        p_pair8 = es_xtb.enter_context(tc.tile_pool(name="p_pair8", bufs=4))
        es_qkv = ExitStack()
        p_xt8 = es_qkv.enter_context(tc.tile_pool(name="p_xt8", bufs=1))
        p_w8 = es_qkv.enter_context(tc.tile_pool(name="p_w8", bufs=3))
        p_vv8 = es_qkv.enter_context(tc.tile_pool(name="p_vv8", bufs=2))
        # right side: FFN pools (reserve order for LIFO closes)
        es_int = ExitStack()
        p_int = es_int.enter_context(tc.tile_pool(name="p_int", bufs=32, side="right"))
        es_aot = ExitStack()
        p_aot = es_aot.enter_context(tc.tile_pool(name="p_aot", bufs=8, side="right"))

        # ---------- DMA in (sync queue), consumption order ----------
        # combined single-DMA loads: [512, N] dram -> [128, 4, N] sbuf view
        def _load4(pool, dram, n, tag):
            big = pool.tile([128, 4 * n], FP8, tag=tag, name=f"big_{tag}")
            nc.sync.dma_start(
                out=big.rearrange("p (j n) -> p j n", j=4),
                in_=dram.rearrange("(j p) n -> p j n", p=128))
            return [big[:, n * j:n * (j + 1)] for j in range(4)]

        xt8 = _load4(p_xt8, xt8_d, 1024, "xt8")
        wv8 = _load4(p_w8, wv8_d, 2048, "wv8")
        wq8 = _load4(p_w8, wq8_d, 2048, "wq8")
        wk8 = _load4(p_w8, wk8_d, 2048, "wk8")

        t_biast = cst.tile([128, 72], F32, tag="t_biast")
        nc.sync.dma_start(out=t_biast, in_=biast_d)
        t_bq, t_bk = t_biast[:, 0:8], t_biast[:, 8:16]
        t_bo = t_biast[:, 16:24]
        t_g1, t_b1 = t_biast[:, 24:32], t_biast[:, 32:40]
        t_bi = t_biast[:, 40:72]
        t_mask = cst.tile([128, ST], F32, tag="t_mask")
        nc.sync.dma_start(out=t_mask, in_=maskb_d)
        t_sel = cst.tile([64, 256], BF16, tag="t_sel")
        nc.sync.dma_start(out=t_sel, in_=sel_d)
        t_sel_a, t_sel_b = t_sel[:, 0:128], t_sel[:, 128:256]
        t_onec = cst.tile([128, 1], BF16, tag="t_onec")
        nc.sync.dma_start(out=t_onec, in_=onec_d)
        t_oner = cst.tile([1, 128], BF16, tag="t_oner")
        nc.sync.dma_start(out=t_oner, in_=oner_d)
        t_bv16 = cst.tile([128, H], BF16, tag="t_bv16")
        nc.sync.dma_start(out=t_bv16, in_=bv16_d.partition_broadcast(128))
        wo8 = _load4(p_wo8, wo8_d, 2048, "wo8")
        xtb_big = p_xtb.tile([128, KT * S], BF16, tag="xtb", name="xtb_big")
        nc.sync.dma_start(
            out=xtb_big.rearrange("p (j n) -> p j n", j=KT),
            in_=xtb_d.rearrange("(j p) n -> p j n", p=128))
        xtb = [xtb_big[:, S * j:S * (j + 1)] for j in range(KT)]
        t_bo2r = cst.tile([128, H], BF16, tag="t_bo2r")
        nc.sync.dma_start(out=t_bo2r, in_=bo2r_d.partition_broadcast(128))
        t_g2r = cst.tile([128, H], BF16, tag="t_g2r")
        nc.sync.dma_start(out=t_g2r, in_=g2r_d.partition_broadcast(128))
        t_b2r = cst.tile([128, H], BF16, tag="t_b2r")
        nc.sync.dma_start(out=t_b2r, in_=b2r_d.partition_broadcast(128))

        ident = cst.tile([128, 128], BF16, tag="ident")
        make_identity(nc, ident)
        t_eps1 = cst.tile([1, 1], F32, tag="t_eps1")
        nc.vector.memset(t_eps1, EPS)
        t_eps128 = cst.tile([128, 1], F32, tag="t_eps128")
        nc.vector.memset(t_eps128, EPS)

        # V'' tiles: [128 kpos, 2(kt pair), 16 heads, 128 (64 V | 64 ones)]
        vv8 = [p_vv8.tile([128, 4096], FP8, tag="vv8", name=f"vv8_{sp}")
               for sp in range(2)]
        for sp in range(2):
            nc.gpsimd.memset(
                vv8[sp].rearrange("p (i h c) -> p i h c", i=2, c=128)[:, :, :, 64:128],
                1.0)

        r3 = lambda t: t.rearrange("p (i n) -> p i n", i=2)

        # ---------- V projection (fp8 DoubleRow, activation-stationary) ----------
        with tc.tile_pool(name="ps_v", bufs=2, space="PSUM") as ps_v:
            for s in range(ST):
                for n in range(2):
                    ps = ps_v.tile([128, 512], F32, tag="psv")
                    for p in range(PR):
                        nc.tensor.matmul(
                            ps, r3(xt8[p])[:, :, 128 * s:128 * (s + 1)],
                            r3(wv8[p])[:, :, 512 * n:512 * (n + 1)],
                            start=(p == 0), stop=(p == PR - 1), perf_mode=DR)
                    nc.vector.tensor_tensor(
                        out=vv8[s // 2].rearrange(
                            "p (i h c) -> p i h c", i=2, c=128)[:, s % 2, 8 * n:8 * n + 8, 0:64],
                        in0=ps.rearrange("p (h c) -> p h c", c=64),
                        in1=t_bv16[:, 512 * n:512 * (n + 1)].rearrange(
                            "p (h c) -> p h c", c=64),
                        op=ALU.add)

        # ---------- Q/K projections (dense PE phase, biases on DVE) ----------
        pair8 = [p_pair8.tile([128, 1024], FP8, tag="pair8", name=f"pair8_{j}")
                 for j in range(PR)]
        with tc.tile_pool(name="p_qt", bufs=8) as p_qt, \
             tc.tile_pool(name="p_kt", bufs=8) as p_kt:
            qt = [p_qt.tile([128, S], BF16, tag="qt", name=f"qt{t}")
                  for t in range(KT)]
            kt = [p_kt.tile([128, S], BF16, tag="kt", name=f"kt{t}")
                  for t in range(KT)]
            with tc.tile_pool(name="ps_qk", bufs=4, space="PSUM") as ps_qk:
                for t in range(KT):
                    ps_q = ps_qk.tile([128, S], F32, tag="psq")
                    for p in range(PR):
                        nc.tensor.matmul(
                            ps_q, r3(wq8[p])[:, :, 128 * t:128 * (t + 1)],
                            r3(xt8[p]), start=(p == 0), stop=(p == PR - 1),
                            perf_mode=DR)
                    nc.scalar.activation(out=qt[t], in_=ps_q, func=AF.Identity,
                                         bias=t_bq[:, t:t + 1], scale=1.0 / WS)
                    ps_k = ps_qk.tile([128, S], F32, tag="psq")
                    for p in range(PR):
                        nc.tensor.matmul(
                            ps_k, r3(wk8[p])[:, :, 128 * t:128 * (t + 1)],
                            r3(xt8[p]), start=(p == 0), stop=(p == PR - 1),
                            perf_mode=DR)
                    nc.scalar.activation(out=kt[t], in_=ps_k, func=AF.Identity,
                                         bias=t_bk[:, t:t + 1], scale=1.0 / WS)

            # ---------- attention (2-head-batched exp, sw-pipelined) ----------
            with tc.tile_pool(name="p_e8", bufs=4) as p_e8, \
                 tc.tile_pool(name="p_ch", bufs=4) as p_ch, \
                 tc.tile_pool(name="p_sums", bufs=4) as p_sums, \
                 tc.tile_pool(name="ps_sc", bufs=2, space="PSUM") as ps_sc, \
                 tc.tile_pool(name="ps_ctx", bufs=2, space="PSUM") as ps_ctx, \
                 tc.tile_pool(name="ps_pk", bufs=2, space="PSUM") as ps_pk:

                def emit_scores(t):
                    # e8c: [kpos, i(kt pair), qA | qB]
                    e8c = [p_e8.tile([128, 2048], FP8, tag="e8",
                                     name=f"e8_{t}_{_i}") for _i in range(2)]
                    for kt_i in range(ST):
                        psc = ps_sc.tile([128, 1024], F32, tag="sc")
                        for hh in range(2):
                            lo, hi = 64 * hh, 64 * hh + 64
                            nc.tensor.matmul(
                                psc[:, 512 * hh:512 * (hh + 1)],
                                kt[t][lo:hi, 128 * kt_i:128 * (kt_i + 1)],
                                qt[t][lo:hi, :], start=True, stop=True)
                        nc.scalar.activation(
                            out=e8c[kt_i // 2].rearrange(
                                "p (i n) -> p i n", i=2)[:, kt_i % 2, :],
                            in_=psc, func=AF.Exp,
                            bias=t_mask[:, kt_i:kt_i + 1], scale=1.0 / 8.0)
                    return e8c

                def emit_ctx(t, e8c):
                    ctxh = {}
                    for hh in range(2):
                        h = 2 * t + hh
                        ps_c = ps_ctx.tile([128, S], F32, tag="ctx")
                        for sp in range(2):
                            nc.tensor.matmul(
                                ps_c,
                                r3(vv8[sp])[:, :, 128 * h:128 * (h + 1)],
                                e8c[sp].rearrange(
                                    "p (i g n) -> p i g n", i=2, n=512)[:, :, hh, :],
                                start=(sp == 0), stop=(sp == 1), perf_mode=DR)
                        sums = p_sums.tile([64, S], F32, tag="sums")
                        nc.vector.tensor_copy(out=sums, in_=ps_c[64:128, :])
                        rinv = p_sums.tile([64, S], F32, tag="sums")
                        nc.vector.reciprocal_approx_fast(out=rinv, in_=sums)
                        ch = p_ch.tile([64, S], BF16, tag="ch")
                        nc.vector.tensor_tensor(out=ch, in0=ps_c[0:64, :],
                                                in1=rinv, op=ALU.mult)
                        ctxh[hh] = ch
                    ps_p = ps_pk.tile([128, S], F32, tag="pk")
                    nc.tensor.matmul(ps_p, t_sel_a, ctxh[0], start=True, stop=False)
                    nc.tensor.matmul(ps_p, t_sel_b, ctxh[1], start=False, stop=True)
                    nc.vector.tensor_copy(
                        out=pair8[t // 2][:, 512 * (t % 2):512 * (t % 2 + 1)],
                        in_=ps_p)

                e8c_prev = emit_scores(0)
                for t in range(1, KT):
                    e8c_cur = emit_scores(t)
                    emit_ctx(t - 1, e8c_prev)
                    e8c_prev = e8c_cur
                emit_ctx(KT - 1, e8c_prev)
        es_qkv.close()

        # ---------- attention output proj + residual + LN1 ----------
        zt = []
        with tc.tile_pool(name="p_zt", bufs=8) as p_zt, \
             tc.tile_pool(name="p_ztmp", bufs=3) as p_ztmp, \
             tc.tile_pool(name="p_sq", bufs=3) as p_sq, \
             tc.tile_pool(name="p_stat", bufs=1) as p_stat, \
             tc.tile_pool(name="p_rep", bufs=1) as p_rep, \
             tc.tile_pool(name="ps_wo", bufs=2, space="PSUM") as ps_wo, \
             tc.tile_pool(name="ps_stat", bufs=2, space="PSUM") as ps_stat, \
             tc.tile_pool(name="ps_rep", bufs=2, space="PSUM") as ps_rep:
            ps_sum = ps_stat.tile([1, S], F32, tag="lnsum")
            ps_sumsq = ps_stat.tile([1, S], F32, tag="lnsum")
            for m in range(KT):
                ps = ps_wo.tile([128, S], F32, tag="wo")
                for p in range(PR):
                    nc.tensor.matmul(ps, r3(wo8[p])[:, :, 128 * m:128 * (m + 1)],
                                     r3(pair8[p]), start=(p == 0),
                                     stop=(p == PR - 1), perf_mode=DR)
                ztmp = p_ztmp.tile([128, S], BF16, tag="ztmp")
                nc.scalar.activation(out=ztmp, in_=ps, func=AF.Identity,
                                     bias=t_bo[:, m:m + 1], scale=1.0 / (WS * WS))
                z = p_zt.tile([128, S], BF16, tag="zt", name=f"zt{m}")
                nc.vector.tensor_tensor(out=z, in0=ztmp, in1=xtb[m], op=ALU.add)
                zt.append(z)
                sq = p_sq.tile([128, S], BF16, tag="sq")
                nc.vector.tensor_tensor(out=sq, in0=z, in1=z, op=ALU.mult)
                nc.tensor.matmul(ps_sum, t_onec, z, start=(m == 0),
                                 stop=(m == KT - 1))
                nc.tensor.matmul(ps_sumsq, t_onec, sq, start=(m == 0),
                                 stop=(m == KT - 1))
            mu_f = p_stat.tile([1, S], F32, tag="mu_f")
            nc.vector.tensor_scalar(out=mu_f, in0=ps_sum, scalar1=1.0 / H,
                                    scalar2=None, op0=ALU.mult)
            mu_bf = p_stat.tile([1, S], BF16, tag="mu_bf")
            nc.vector.tensor_copy(out=mu_bf, in_=mu_f)
            ex2 = p_stat.tile([1, S], F32, tag="ex2")
            nc.vector.tensor_scalar(out=ex2, in0=ps_sumsq, scalar1=1.0 / H,
                                    scalar2=None, op0=ALU.mult)
            mu2 = p_stat.tile([1, S], F32, tag="mu2")
            nc.vector.tensor_tensor(out=mu2, in0=mu_f, in1=mu_f, op=ALU.mult)
            var = p_stat.tile([1, S], F32, tag="var")
            nc.vector.tensor_tensor(out=var, in0=ex2, in1=mu2, op=ALU.subtract)
            sd = p_stat.tile([1, S], F32, tag="sd")
            nc.scalar.activation(out=sd, in_=var, func=AF.Sqrt, bias=t_eps1,
                                 scale=1.0)
            rstd_f = p_stat.tile([1, S], F32, tag="rstd_f")
            nc.vector.reciprocal_approx_fast(out=rstd_f, in_=sd)
            rstd_bf = p_stat.tile([1, S], BF16, tag="rstd_bf")
            nc.vector.tensor_copy(out=rstd_bf, in_=rstd_f)
            ps_mu = ps_rep.tile([128, S], F32, tag="murep")
            nc.tensor.matmul(ps_mu, t_oner, mu_bf, start=True, stop=True)
            ps_rstd = ps_rep.tile([128, S], F32, tag="murep")
            nc.tensor.matmul(ps_rstd, t_oner, rstd_bf, start=True, stop=True)
            murep = p_rep.tile([128, S], BF16, tag="murep_sb")
            nc.vector.tensor_copy(out=murep, in_=ps_mu)
            rstdrep = p_rep.tile([128, S], BF16, tag="rstdrep_sb")
            nc.vector.tensor_copy(out=rstdrep, in_=ps_rstd)
            aot = [p_aot.tile([128, S], BF16, tag="aot", name=f"aot{m}")
                   for m in range(KT)]
            for m in range(KT):
                t1 = p_sq.tile([128, S], BF16, tag="t1")
                nc.vector.tensor_tensor(out=t1, in0=zt[m], in1=murep,
                                        op=ALU.subtract)
                t2 = p_sq.tile([128, S], BF16, tag="t2")
                nc.vector.tensor_tensor(out=t2, in0=t1, in1=rstdrep, op=ALU.mult)
                nc.vector.tensor_scalar(out=aot[m], in0=t2,
                                        scalar1=t_g1[:, m:m + 1],
                                        scalar2=t_b1[:, m:m + 1],
                                        op0=ALU.mult, op1=ALU.add)
        es_xtb.close()

        # ---------- aot transposes (normal-layout residual for FFN2) ----------
        p_aotbo = es.enter_context(tc.tile_pool(name="p_aotbo", bufs=4))
        aotbo = [p_aotbo.tile([128, H], BF16, tag="aotbo", name=f"aotbo{s}")
                 for s in range(ST)]
        with tc.tile_pool(name="ps_tp", bufs=3, space="PSUM") as ps_tp:
            for m in range(KT):
                for s in range(ST):
                    pt = ps_tp.tile([128, 128], BF16, tag="tp")
                    nc.tensor.transpose(pt, aot[m][:, 128 * s:128 * (s + 1)], ident)
                    nc.vector.tensor_tensor(
                        out=aotbo[s][:, 128 * m:128 * (m + 1)], in0=pt,
                        in1=t_bo2r[:, 128 * m:128 * (m + 1)], op=ALU.add)

        # ---------- FFN1 (bf16, wi stationary) ----------
        # wo2 fully resident for FFN2 (DMA'd during FFN1 on the gpsimd queue)
        p_wo2r = es.enter_context(tc.tile_pool(name="p_wo2r", bufs=1))
        wo2r_big = p_wo2r.tile([128, FT * H], BF16, tag="wo2r", name="wo2r_big")
        nc.gpsimd.dma_start(
            out=wo2r_big.rearrange("p (j n) -> p j n", j=FT),
            in_=wo2_d.rearrange("(j p) n -> p j n", p=128))
        wo2r = [wo2r_big[:, H * f:H * (f + 1)] for f in range(FT)]
        intert = [p_int.tile([128, S], BF16, tag="intert", name=f"int{f}")
                  for f in range(FT)]
        with tc.tile_pool(name="p_wi", bufs=16) as p_wi, \
             tc.tile_pool(name="ps_f1", bufs=3, space="PSUM") as ps_f1:
            for fb in range(FT // 4):
                wic = []
                for k in range(KT):
                    c = p_wi.tile([128, 512], BF16, tag="wi")
                    nc.sync.dma_start(
                        out=c, in_=wi_d[128 * k:128 * (k + 1),
                                        512 * fb:512 * (fb + 1)])
                    wic.append(c)
                for fi in range(4):
                    f = 4 * fb + fi
                    ps = ps_f1.tile([128, S], F32, tag="f1")
                    for k in range(KT):
                        nc.tensor.matmul(ps, wic[k][:, 128 * fi:128 * (fi + 1)],
                                         aot[k], start=(k == 0),
                                         stop=(k == KT - 1))
                    nc.scalar.activation(out=intert[f], in_=ps, func=AF.Gelu,
                                         bias=t_bi[:, f:f + 1], scale=1.0)
        es_aot.close()

        # ---------- FFN2 (bf16, activation-stationary, s-outer) + LN2 ----------
        with tc.tile_pool(name="p_z2", bufs=3) as p_z2, \
             tc.tile_pool(name="p_sq2", bufs=2) as p_sq2, \
             tc.tile_pool(name="p_st2", bufs=4) as p_st2, \
             tc.tile_pool(name="p_y", bufs=3) as p_y, \
             tc.tile_pool(name="ps_f2", bufs=4, space="PSUM") as ps_f2:
            for s in range(ST):
                ps_a = ps_f2.tile([128, 512], F32, tag="f2")
                ps_b = ps_f2.tile([128, 512], F32, tag="f2")
                for f in range(FT):
                    lhsT = intert[f][:, 128 * s:128 * (s + 1)]
                    nc.tensor.matmul(ps_a, lhsT, wo2r[f][:, 0:512],
                                     start=(f == 0), stop=(f == FT - 1))
                    nc.tensor.matmul(ps_b, lhsT, wo2r[f][:, 512:1024],
                                     start=(f == 0), stop=(f == FT - 1))
                z2 = p_z2.tile([128, H], BF16, tag="z2")
                for hh, pso in ((0, ps_a), (1, ps_b)):
                    nc.vector.tensor_tensor(
                        out=z2[:, 512 * hh:512 * (hh + 1)], in0=pso,
                        in1=aotbo[s][:, 512 * hh:512 * (hh + 1)], op=ALU.add)
                s1 = p_st2.tile([128, 1], F32, tag="s1")
                nc.vector.tensor_reduce(out=s1, in_=z2, axis=mybir.AxisListType.X,
                                        op=ALU.add)
                sq2 = p_sq2.tile([128, H], BF16, tag="sq2")
                s2 = p_st2.tile([128, 1], F32, tag="s2")
                nc.scalar.activation(out=sq2, in_=z2, func=AF.Square,
                                     accum_out=s2)
                mu = p_st2.tile([128, 1], F32, tag="mu")
                nc.vector.tensor_scalar(out=mu, in0=s1, scalar1=1.0 / H,
                                        scalar2=None, op0=ALU.mult)
                ex2b = p_st2.tile([128, 1], F32, tag="ex2b")
                nc.vector.tensor_scalar(out=ex2b, in0=s2, scalar1=1.0 / H,
                                        scalar2=None, op0=ALU.mult)
                mu2b = p_st2.tile([128, 1], F32, tag="mu2b")
                nc.vector.tensor_tensor(out=mu2b, in0=mu, in1=mu, op=ALU.mult)
                varb = p_st2.tile([128, 1], F32, tag="varb")
                nc.vector.tensor_tensor(out=varb, in0=ex2b, in1=mu2b,
                                        op=ALU.subtract)
                sdb = p_st2.tile([128, 1], F32, tag="sdb")
                nc.scalar.activation(out=sdb, in_=varb, func=AF.Sqrt,
                                     bias=t_eps128, scale=1.0)
                rstdb = p_st2.tile([128, 1], F32, tag="rstdb")
                nc.vector.reciprocal(out=rstdb, in_=sdb)
                nmu = p_st2.tile([128, 1], F32, tag="nmu")
                nc.vector.tensor_tensor(out=nmu, in0=mu, in1=rstdb, op=ALU.mult)
                nb = p_st2.tile([128, 1], F32, tag="nb")
                nc.vector.tensor_scalar(out=nb, in0=nmu, scalar1=-1.0,
                                        scalar2=None, op0=ALU.mult)
                y1 = p_y.tile([128, H], BF16, tag="y1")
                nc.scalar.activation(out=y1, in_=z2, func=AF.Identity,
                                     bias=nb, scale=rstdb)
                y2 = p_y.tile([128, H], BF16, tag="y2")
                nc.vector.tensor_tensor(out=y2, in0=y1, in1=t_g2r, op=ALU.mult)
                y3 = p_y.tile([128, H], BF16, tag="y3")
                nc.vector.tensor_tensor(out=y3, in0=y2, in1=t_b2r, op=ALU.add)
                nc.sync.dma_start(out=out_d[128 * s:128 * (s + 1), :], in_=y3)
        es_int.close()

    nc.compile()
    return nc


def _get_nc():
    if "nc" not in _CACHE:
        _CACHE["nc"] = _build()
    return _CACHE["nc"]


def _perpart(v):
    # [n*128] -> [128, n] with vT[p, t] = v[t*128 + p]
    v = np.asarray(v, np.float32)
    return np.ascontiguousarray(v.reshape(-1, 128).T)


def _pack_pairs(w, scale, dtype):
    # w [K, N] -> [K/2, 2N] fp8: row (128p + k) holds [i in {0,1}, n] with
    # value w[(2p+i)*128 + k, n] * scale
    w = np.asarray(w, np.float32) * scale
    K, N = w.shape
    t = w.reshape(K // 256, 2, 128, N).transpose(0, 2, 1, 3)
    return np.ascontiguousarray(t.reshape(K // 2, 2 * N)).astype(dtype)


def _shared_inputs(inp):
    f8 = ml_dtypes.float8_e4m3
    bf = ml_dtypes.bfloat16
    f = np.float32
    biast = np.concatenate(
        [_perpart(inp["bq"]), _perpart(inp["bk"]), _perpart(inp["bo"]),
         _perpart(inp["ln1_g"]), _perpart(inp["ln1_b"]), _perpart(inp["bi"])],
        axis=1).astype(f)
    sel = np.zeros((64, 256), f)
    sel[:, 0:64] = np.eye(64)
    sel[:, 192:256] = np.eye(64)
    return {
        "wq8": _pack_pairs(inp["wq"], WS, f8),
        "wk8": _pack_pairs(inp["wk"], WS, f8),
        "wv8": _pack_pairs(inp["wv"], WS, f8),
        "wo8": _pack_pairs(inp["wo"], WS, f8),
        "wi": np.ascontiguousarray(np.asarray(inp["wi"], f)).astype(bf),
        "wo2": np.ascontiguousarray(np.asarray(inp["wo2"], f)).astype(bf),
        "biast": biast,
        "bv16": (np.asarray(inp["bv"], f) * WS).reshape(1, H).astype(bf),
        "bo2r": np.asarray(inp["bo2"], f).reshape(1, H).astype(bf),
        "g2r": np.asarray(inp["ln2_g"], f).reshape(1, H).astype(bf),
        "b2r": np.asarray(inp["ln2_b"], f).reshape(1, H).astype(bf),
        "sel": sel.astype(bf),
        "onec": np.ones((128, 1), f).astype(bf),
        "oner": np.ones((1, 128), f).astype(bf),
    }


def _core_inputs(shared, hs, am, b):
    f8 = ml_dtypes.float8_e4m3
    bf = ml_dtypes.bfloat16
    xT = np.ascontiguousarray(hs[b].T)  # [H, S] fp32
    return dict(
        shared,
        xt8=_pack_pairs(xT, 1.0, f8),
        xtb=xT.astype(bf),
        maskb=(_perpart(am[b]) + EXP_SHIFT).astype(np.float32),
    )


def kernel(hidden_states, attention_mask, wq, bq, wk, bk, wv, bv,
           wo, bo, ln1_g, ln1_b, wi, bi, wo2, bo2, ln2_g, ln2_b):
    nc = _get_nc()
    f = np.float32
    shared = _shared_inputs({
        "wq": wq, "wk": wk, "wv": wv, "wo": wo, "wi": wi, "wo2": wo2,
        "bq": bq, "bk": bk, "bv": bv, "bo": bo, "bi": bi, "bo2": bo2,
        "ln1_g": ln1_g, "ln1_b": ln1_b, "ln2_g": ln2_g, "ln2_b": ln2_b,
    })
    hs = np.ascontiguousarray(hidden_states, f)
    am = np.ascontiguousarray(attention_mask, f).reshape(B, S)
    in_maps = [_core_inputs(shared, hs, am, b) for b in range(B)]
    res = bass_utils.run_bass_kernel_spmd(nc, in_maps, core_ids=list(range(B)),
                                          trace=False)
    return np.stack([res.results[b]["out"].astype(f) for b in range(B)])


# revision 21
# speedup vs baseline: 1.3307x; 1.0330x over previous
"""Trainium2 Bass kernel for a single RoBERTa encoder layer.

Problem: B=8, S=512, H=1024, 16 heads (d=64), FF=4096, fp32 in/out, eval.

Strategy: data-parallel over batch (one batch element per core, 8 cores).
Per core, activations flow in a transposed "feature-on-partitions" layout.
Matmul dtypes: fp8e4 (e4m3) with DoubleRow perf mode (2x PE throughput)
for the QKV projections, the probs@V context matmul and the attention
output projection; bf16 (1 cyc/row) for scores, FFN1 and FFN2. Empirically
(see fp8 experiment) this config lands at ~7e-3 relative error vs the 2e-2
gate; fp8 in the FFN would exceed the budget.

Scaling tricks:
  - weights pre-scaled x16 before fp8 cast (avoids e4m3 subnormals),
    un-scaled for free via the ACT bias/scale path off PSUM.
  - exp() output scaled by exp(-2.34) so probs fit e4m3 nicely; softmax
    normalization (ones-rows trick inside the V'' matrix) cancels it.
  - host pre-transposes X and pre-packs all DoubleRow operand layouts.

Layout per core:
  xt (bf16, [h=128 x 8, tok 512])   transposed input, residual 1
  xt8 (fp8 pairs)                   QKV moving / V stationary operand
  qt/kt [feat 128, tok 512] bf16 -> scoresT[kpos, q] via 64-row stationary
  e8 = fp8(exp(scores/8 + mask - 2.34)) pairs -> ctx via DoubleRow with
    V''=[16(V+bv) | ones] -> ctx rows 0:64, sumexp rows 64:128
  pair8 = fp8(16*ctx) packed head pairs -> wo DoubleRow -> +bo +xt -> LN1
  FFN1 bf16 (wi stationary) -> gelu -> interT
  FFN2 bf16 activation-stationary (interT tiles) -> [tok, col] + aot^T
    + bo2 residual -> LN2 in normal layout -> bf16 out (host casts fp32)
"""
import math

import numpy as np
import ml_dtypes

import concourse.bass as bass
import concourse.mybir as mybir
import concourse.tile as tile
from concourse import bacc
from concourse import bass_utils
from concourse.masks import make_identity

F32 = mybir.dt.float32
BF16 = mybir.dt.bfloat16
FP8 = mybir.dt.float8e4
AF = mybir.ActivationFunctionType
ALU = mybir.AluOpType
DR = mybir.MatmulPerfMode.DoubleRow

B, S, H, NH, HD, FF = 8, 512, 1024, 16, 64, 4096
KT = H // 128       # 8 hidden k-tiles
PR = KT // 2        # 4 k-tile pairs (fp8 DoubleRow)
ST = S // 128       # 4 token tiles
FT = FF // 128      # 32 ff tiles
EPS = 1e-5
WS = 16.0           # weight prescale before fp8 cast
EXP_SHIFT = math.log(64.0) - 6.5   # folded into exp bias; cancels in softmax

_CACHE = {}


def _build():
    nc = bacc.Bacc("TRN2", target_bir_lowering=False, debug=False,
                   enable_asserts=True, num_devices=B)

    def din(name, shape, dt):
        return nc.dram_tensor(name, shape, dt, kind="ExternalInput").ap()

    # per-core inputs (host pre-packed)
    xt8_d = din("xt8", [512, 1024], FP8)        # pair-packed fp8 X^T
    xtb_d = din("xtb", [H, S], BF16)            # X^T bf16
    maskb_d = din("maskb", [128, ST], F32)      # per-kpos mask + EXP_SHIFT
    # shared weights
    wq8_d = din("wq8", [512, 2048], FP8)
    wk8_d = din("wk8", [512, 2048], FP8)
    wv8_d = din("wv8", [512, 2048], FP8)
    wo8_d = din("wo8", [512, 2048], FP8)
    wi_d = din("wi", [H, FF], BF16)
    wo2_d = din("wo2", [FF, H], BF16)
    # biast: bq bk bo g1 b1 (8 cols each) + bi (32) = 72 cols fp32
    biast_d = din("biast", [128, 72], F32)
    bv16_d = din("bv16", [1, H], BF16)          # 16*bv
    bo2r_d = din("bo2r", [1, H], BF16)
    g2r_d = din("g2r", [1, H], BF16)
    b2r_d = din("b2r", [1, H], BF16)
    ones8_d = din("ones8", [128, 512], FP8)     # [1|0] / [0|1] sums stationary
    onec_d = din("onec", [128, 1], BF16)
    oner_d = din("oner", [1, 128], mybir.dt.float32r)
    out_d = nc.dram_tensor("out", [S, H], BF16, kind="ExternalOutput").ap()

    from contextlib import ExitStack
    es = ExitStack()
    with tile.TileContext(nc) as tc, es:
        # ---------- long-lived pools ----------
        cst = es.enter_context(tc.tile_pool(name="cst", bufs=1))
        es_xtb = ExitStack()
        p_xtb = es_xtb.enter_context(tc.tile_pool(name="p_xtb", bufs=1))
        p_wo8 = es_xtb.enter_context(tc.tile_pool(name="p_wo8", bufs=1))
        p_pair8 = es_xtb.enter_context(tc.tile_pool(name="p_pair8", bufs=4))
        es_qkv = ExitStack()
        p_xt8 = es_qkv.enter_context(tc.tile_pool(name="p_xt8", bufs=1))
        p_w8 = es_qkv.enter_context(tc.tile_pool(name="p_w8", bufs=3))
        p_vv8 = es_qkv.enter_context(tc.tile_pool(name="p_vv8", bufs=2))
        # right side: FFN pools (reserve order for LIFO closes)
        es_int = ExitStack()
        p_int = es_int.enter_context(tc.tile_pool(name="p_int", bufs=32, side="right"))
        es_aot = ExitStack()
        p_aot = es_aot.enter_context(tc.tile_pool(name="p_aot", bufs=8, side="right"))

        # ---------- DMA in (sync queue), consumption order ----------
        # combined single-DMA loads: [512, N] dram -> [128, 4, N] sbuf view
        def _load4(pool, dram, n, tag):
            big = pool.tile([128, 4 * n], FP8, tag=tag, name=f"big_{tag}")
            nc.sync.dma_start(
                out=big.rearrange("p (j n) -> p j n", j=4),
                in_=dram.rearrange("(j p) n -> p j n", p=128))
            return [big[:, n * j:n * (j + 1)] for j in range(4)]

        xt8 = _load4(p_xt8, xt8_d, 1024, "xt8")
        wv8 = _load4(p_w8, wv8_d, 2048, "wv8")
        wq8 = _load4(p_w8, wq8_d, 2048, "wq8")
        wk8 = _load4(p_w8, wk8_d, 2048, "wk8")

        t_biast = cst.tile([128, 72], F32, tag="t_biast")
        nc.sync.dma_start(out=t_biast, in_=biast_d)
        t_bq, t_bk = t_biast[:, 0:8], t_biast[:, 8:16]
        t_bo = t_biast[:, 16:24]
        t_g1, t_b1 = t_biast[:, 24:32], t_biast[:, 32:40]
        t_bi = t_biast[:, 40:72]
        t_mask = cst.tile([128, ST], F32, tag="t_mask")
        nc.sync.dma_start(out=t_mask, in_=maskb_d)
        t_onec = cst.tile([128, 1], BF16, tag="t_onec")
        nc.sync.dma_start(out=t_onec, in_=onec_d)
        t_oner = cst.tile([1, 128], mybir.dt.float32r, tag="t_oner")
        nc.sync.dma_start(out=t_oner, in_=oner_d)
        t_bv16 = cst.tile([128, H], BF16, tag="t_bv16")
        nc.sync.dma_start(out=t_bv16, in_=bv16_d.partition_broadcast(128))
        wo8 = _load4(p_wo8, wo8_d, 2048, "wo8")
        xtb_big = p_xtb.tile([128, KT * S], BF16, tag="xtb", name="xtb_big")
        nc.sync.dma_start(
            out=xtb_big.rearrange("p (j n) -> p j n", j=KT),
            in_=xtb_d.rearrange("(j p) n -> p j n", p=128))
        xtb = [xtb_big[:, S * j:S * (j + 1)] for j in range(KT)]
        t_bo2r = cst.tile([128, H], BF16, tag="t_bo2r")
        nc.sync.dma_start(out=t_bo2r, in_=bo2r_d.partition_broadcast(128))
        t_g2r = cst.tile([128, H], BF16, tag="t_g2r")
        nc.sync.dma_start(out=t_g2r, in_=g2r_d.partition_broadcast(128))
        t_b2r = cst.tile([128, H], BF16, tag="t_b2r")
        nc.sync.dma_start(out=t_b2r, in_=b2r_d.partition_broadcast(128))

        ident = cst.tile([128, 128], BF16, tag="ident")
        make_identity(nc, ident)
        t_eps1 = cst.tile([1, 1], F32, tag="t_eps1")
        nc.vector.memset(t_eps1, EPS)
        t_eps128 = cst.tile([128, 1], F32, tag="t_eps128")
        nc.vector.memset(t_eps128, EPS)

        # V'' tiles: [128 kpos, i(2), pair(8), 256 = VA(64)|0(128)|VB(64)]
        vv8 = [p_vv8.tile([128, 4096], FP8, tag="vv8", name=f"vv8_{sp}")
               for sp in range(2)]
        for sp in range(2):
            nc.gpsimd.memset(vv8[sp], 0.0)
        t_ones8 = cst.tile([128, 512], FP8, tag="t_ones8")
        nc.sync.dma_start(out=t_ones8, in_=ones8_d)

        r3 = lambda t: t.rearrange("p (i n) -> p i n", i=2)

        # ---------- V projection (fp8 DoubleRow, activation-stationary) ----------
        with tc.tile_pool(name="ps_v", bufs=2, space="PSUM") as ps_v:
            for s in range(ST):
                for n in range(2):
                    ps = ps_v.tile([128, 512], F32, tag="psv")
                    for p in range(PR):
                        nc.tensor.matmul(
                            ps, r3(xt8[p])[:, :, 128 * s:128 * (s + 1)],
                            r3(wv8[p])[:, :, 512 * n:512 * (n + 1)],
                            start=(p == 0), stop=(p == PR - 1), perf_mode=DR)
                    vvw = vv8[s // 2].rearrange(
                        "p (i t c) -> p i t c", i=2, c=256)
                    psw = ps.rearrange("p (t two c) -> p t two c", two=2, c=64)
                    bvw = t_bv16[:, 512 * n:512 * (n + 1)].rearrange(
                        "p (t two c) -> p t two c", two=2, c=64)
                    nc.vector.tensor_tensor(
                        out=vvw[:, s % 2, 4 * n:4 * n + 4, 0:64],
                        in0=psw[:, :, 0, :], in1=bvw[:, :, 0, :], op=ALU.add)
                    nc.vector.tensor_tensor(
                        out=vvw[:, s % 2, 4 * n:4 * n + 4, 192:256],
                        in0=psw[:, :, 1, :], in1=bvw[:, :, 1, :], op=ALU.add)

        # ---------- Q/K projections (dense PE phase, biases on DVE) ----------
        pair8 = [p_pair8.tile([128, 1024], FP8, tag="pair8", name=f"pair8_{j}")
                 for j in range(PR)]
        with tc.tile_pool(name="p_qt", bufs=8) as p_qt, \
             tc.tile_pool(name="p_kt", bufs=8) as p_kt:
            qt = [p_qt.tile([128, S], BF16, tag="qt", name=f"qt{t}")
                  for t in range(KT)]
            kt = [p_kt.tile([128, S], BF16, tag="kt", name=f"kt{t}")
                  for t in range(KT)]
            with tc.tile_pool(name="ps_qk", bufs=4, space="PSUM") as ps_qk:
                for t in range(KT):
                    ps_q = ps_qk.tile([128, S], F32, tag="psq")
                    for p in range(PR):
                        nc.tensor.matmul(
                            ps_q, r3(wq8[p])[:, :, 128 * t:128 * (t + 1)],
                            r3(xt8[p]), start=(p == 0), stop=(p == PR - 1),
                            perf_mode=DR)
                    nc.scalar.activation(out=qt[t], in_=ps_q, func=AF.Identity,
                                         bias=t_bq[:, t:t + 1], scale=1.0 / WS)
                    ps_k = ps_qk.tile([128, S], F32, tag="psq")
                    for p in range(PR):
                        nc.tensor.matmul(
                            ps_k, r3(wk8[p])[:, :, 128 * t:128 * (t + 1)],
                            r3(xt8[p]), start=(p == 0), stop=(p == PR - 1),
                            perf_mode=DR)
                    nc.scalar.activation(out=kt[t], in_=ps_k, func=AF.Identity,
                                         bias=t_bk[:, t:t + 1], scale=1.0 / WS)

            # ---------- attention (2-head-batched exp, sw-pipelined) ----------
            with tc.tile_pool(name="p_e8", bufs=4) as p_e8, \
                 tc.tile_pool(name="p_sums", bufs=4) as p_sums, \
                 tc.tile_pool(name="ps_sc", bufs=2, space="PSUM") as ps_sc, \
                 tc.tile_pool(name="ps_ctx", bufs=2, space="PSUM") as ps_ctx, \
                 tc.tile_pool(name="ps_pk", bufs=2, space="PSUM") as ps_pk:

                def emit_scores(t):
                    # e8c: [kpos, i(kt pair), qA | qB]
                    e8c = [p_e8.tile([128, 2048], FP8, tag="e8",
                                     name=f"e8_{t}_{_i}") for _i in range(2)]
                    for kt_i in range(ST):
                        psc = ps_sc.tile([128, 1024], F32, tag="sc")
                        for hh in range(2):
                            lo, hi = 64 * hh, 64 * hh + 64
                            nc.tensor.matmul(
                                psc[:, 512 * hh:512 * (hh + 1)],
                                kt[t][lo:hi, 128 * kt_i:128 * (kt_i + 1)],
                                qt[t][lo:hi, :], start=True, stop=True)
                        nc.scalar.activation(
                            out=e8c[kt_i // 2].rearrange(
                                "p (i n) -> p i n", i=2)[:, kt_i % 2, :],
                            in_=psc, func=AF.Exp,
                            bias=t_mask[:, kt_i:kt_i + 1], scale=1.0 / 8.0)
                    return e8c

                ones8r = t_ones8.rearrange("p (g i c) -> p g i c", g=2, c=128)

                def emit_ctx(t, e8c):
                    # heads A/B stacked in one [128,512] psum via zero-padded
                    # stationaries [VA|0] / [0|VB]; sums likewise with [1|0]/[0|1]
                    ps_c = ps_ctx.tile([128, S], F32, tag="ctx")
                    ps_s2 = ps_pk.tile([128, S], F32, tag="pk")
                    k = 0
                    for hh in range(2):
                        for sp in range(2):
                            mv = e8c[sp].rearrange(
                                "p (i g n) -> p i g n", i=2, n=512)[:, :, hh, :]
                            st = vv8[sp].rearrange(
                                "p (i t c) -> p i t c", i=2, c=256)[
                                    :, :, t, 128 * hh:128 * hh + 128]
                            nc.tensor.matmul(ps_c, st, mv, start=(k == 0),
                                             stop=(k == 3), perf_mode=DR)
                            nc.tensor.matmul(ps_s2, ones8r[:, hh, :, :], mv,
                                             start=(k == 0), stop=(k == 3),
                                             perf_mode=DR)
                            k += 1
                    sums = p_sums.tile([128, S], F32, tag="sums")
                    nc.vector.tensor_copy(out=sums, in_=ps_s2)
                    rinv = p_sums.tile([128, S], F32, tag="sums")
                    nc.vector.reciprocal_approx_fast(out=rinv, in_=sums)
                    nc.vector.tensor_tensor(
                        out=pair8[t // 2][:, 512 * (t % 2):512 * (t % 2 + 1)],
                        in0=ps_c, in1=rinv, op=ALU.mult)

                e8c_prev = emit_scores(0)
                for t in range(1, KT):
                    e8c_cur = emit_scores(t)
                    emit_ctx(t - 1, e8c_prev)
                    e8c_prev = e8c_cur
                emit_ctx(KT - 1, e8c_prev)
        es_qkv.close()

        # ---------- attention output proj + residual + LN1 ----------
        zt = []
        with tc.tile_pool(name="p_zt", bufs=8) as p_zt, \
             tc.tile_pool(name="p_ztmp", bufs=3) as p_ztmp, \
             tc.tile_pool(name="p_sq", bufs=3) as p_sq, \
             tc.tile_pool(name="p_stat", bufs=1) as p_stat, \
             tc.tile_pool(name="p_rep", bufs=1) as p_rep, \
             tc.tile_pool(name="ps_wo", bufs=2, space="PSUM") as ps_wo, \
             tc.tile_pool(name="ps_stat", bufs=2, space="PSUM") as ps_stat, \
             tc.tile_pool(name="ps_rep", bufs=2, space="PSUM") as ps_rep:
            ps_sum = ps_stat.tile([1, S], F32, tag="lnsum")
            ps_sumsq = ps_stat.tile([1, S], F32, tag="lnsum")
            for m in range(KT):
                ps = ps_wo.tile([128, S], F32, tag="wo")
                for p in range(PR):
                    nc.tensor.matmul(ps, r3(wo8[p])[:, :, 128 * m:128 * (m + 1)],
                                     r3(pair8[p]), start=(p == 0),
                                     stop=(p == PR - 1), perf_mode=DR)
                ztmp = p_ztmp.tile([128, S], BF16, tag="ztmp")
                nc.scalar.activation(out=ztmp, in_=ps, func=AF.Identity,
                                     bias=t_bo[:, m:m + 1], scale=1.0 / (WS * WS))
                z = p_zt.tile([128, S], BF16, tag="zt", name=f"zt{m}")
                nc.vector.tensor_tensor(out=z, in0=ztmp, in1=xtb[m], op=ALU.add)
                zt.append(z)
                sq = p_sq.tile([128, S], BF16, tag="sq")
                nc.vector.tensor_tensor(out=sq, in0=z, in1=z, op=ALU.mult)
                nc.tensor.matmul(ps_sum, t_onec, z, start=(m == 0),
                                 stop=(m == KT - 1))
                nc.tensor.matmul(ps_sumsq, t_onec, sq, start=(m == 0),
                                 stop=(m == KT - 1))
            F32R = mybir.dt.float32r
            mu_f = p_stat.tile([1, S], F32R, tag="mu_f")
            nc.vector.tensor_scalar(out=mu_f, in0=ps_sum, scalar1=1.0 / H,
                                    scalar2=None, op0=ALU.mult)
            ex2 = p_stat.tile([1, S], F32, tag="ex2")
            nc.vector.tensor_scalar(out=ex2, in0=ps_sumsq, scalar1=1.0 / H,
                                    scalar2=None, op0=ALU.mult)
            mu2 = p_stat.tile([1, S], F32, tag="mu2")
            nc.gpsimd.tensor_tensor(out=mu2, in0=mu_f, in1=mu_f, op=ALU.mult)
            var = p_stat.tile([1, S], F32, tag="var")
            nc.vector.tensor_tensor(out=var, in0=ex2, in1=mu2, op=ALU.subtract)
            sd = p_stat.tile([1, S], F32, tag="sd")
            nc.scalar.activation(out=sd, in_=var, func=AF.Sqrt, bias=t_eps1,
                                 scale=1.0)
            rstd_f = p_stat.tile([1, S], F32, tag="rstd_f")
            nc.vector.reciprocal_approx_fast(out=rstd_f, in_=sd)
            rstd_r = p_stat.tile([1, S], F32R, tag="rstd_r")
            nc.scalar.activation(out=rstd_r, in_=rstd_f, func=AF.Identity)
            ps_mu = ps_rep.tile([128, S], F32, tag="murep")
            nc.tensor.matmul(ps_mu, t_oner, mu_f,
                             start=True, stop=True)
            ps_rstd = ps_rep.tile([128, S], F32, tag="murep")
            nc.tensor.matmul(ps_rstd, t_oner, rstd_r,
                             start=True, stop=True)
            murep = p_rep.tile([128, S], BF16, tag="murep_sb")
            nc.vector.tensor_copy(out=murep, in_=ps_mu)
            rstdrep = p_rep.tile([128, S], BF16, tag="rstdrep_sb")
            nc.vector.tensor_copy(out=rstdrep, in_=ps_rstd)
            aot = [p_aot.tile([128, S], BF16, tag="aot", name=f"aot{m}")
                   for m in range(KT)]
            for m in range(KT):
                t1 = p_sq.tile([128, S], BF16, tag="t1")
                nc.vector.tensor_tensor(out=t1, in0=zt[m], in1=murep,
                                        op=ALU.subtract)
                t2 = p_sq.tile([128, S], BF16, tag="t2")
                nc.vector.tensor_tensor(out=t2, in0=t1, in1=rstdrep, op=ALU.mult)
                nc.vector.tensor_scalar(out=aot[m], in0=t2,
                                        scalar1=t_g1[:, m:m + 1],
                                        scalar2=t_b1[:, m:m + 1],
                                        op0=ALU.mult, op1=ALU.add)
        es_xtb.close()

        # ---------- aot transposes (normal-layout residual for FFN2) ----------
        p_aotbo = es.enter_context(tc.tile_pool(name="p_aotbo", bufs=4))
        aotbo = [p_aotbo.tile([128, H], BF16, tag="aotbo", name=f"aotbo{s}")
                 for s in range(ST)]
        with tc.tile_pool(name="ps_tp", bufs=3, space="PSUM") as ps_tp:
            for m in range(KT):
                for s in range(ST):
                    pt = ps_tp.tile([128, 128], BF16, tag="tp")
                    nc.tensor.transpose(pt, aot[m][:, 128 * s:128 * (s + 1)], ident)
                    nc.vector.tensor_tensor(
                        out=aotbo[s][:, 128 * m:128 * (m + 1)], in0=pt,
                        in1=t_bo2r[:, 128 * m:128 * (m + 1)], op=ALU.add)

        # ---------- FFN1 (bf16, wi stationary) ----------
        # wo2 fully resident for FFN2 (DMA'd during FFN1 on the gpsimd queue)
        p_wo2r = es.enter_context(tc.tile_pool(name="p_wo2r", bufs=1))
        wo2r_big = p_wo2r.tile([128, FT * H], BF16, tag="wo2r", name="wo2r_big")
        nc.gpsimd.dma_start(
            out=wo2r_big.rearrange("p (j n) -> p j n", j=FT),
            in_=wo2_d.rearrange("(j p) n -> p j n", p=128))
        wo2r = [wo2r_big[:, H * f:H * (f + 1)] for f in range(FT)]
        intert = [p_int.tile([128, S], BF16, tag="intert", name=f"int{f}")
                  for f in range(FT)]
        with tc.tile_pool(name="p_wi", bufs=16) as p_wi, \
             tc.tile_pool(name="ps_f1", bufs=3, space="PSUM") as ps_f1:
            for fb in range(FT // 4):
                wic = []
                for k in range(KT):
                    c = p_wi.tile([128, 512], BF16, tag="wi")
                    nc.sync.dma_start(
                        out=c, in_=wi_d[128 * k:128 * (k + 1),
                                        512 * fb:512 * (fb + 1)])
                    wic.append(c)
                for fi in range(4):
                    f = 4 * fb + fi
                    ps = ps_f1.tile([128, S], F32, tag="f1")
                    for k in range(KT):
                        nc.tensor.matmul(ps, wic[k][:, 128 * fi:128 * (fi + 1)],
                                         aot[k], start=(k == 0),
                                         stop=(k == KT - 1))
                    nc.scalar.activation(out=intert[f], in_=ps, func=AF.Gelu,
                                         bias=t_bi[:, f:f + 1], scale=1.0)
        es_aot.close()

        # ---------- FFN2 (bf16, activation-stationary, s-outer) + LN2 ----------
        with tc.tile_pool(name="p_z2", bufs=3) as p_z2, \
             tc.tile_pool(name="p_sq2", bufs=2) as p_sq2, \
             tc.tile_pool(name="p_st2", bufs=4) as p_st2, \
             tc.tile_pool(name="p_y", bufs=3) as p_y, \
             tc.tile_pool(name="ps_f2", bufs=4, space="PSUM") as ps_f2:
            for s in range(ST):
                ps_a = ps_f2.tile([128, 512], F32, tag="f2")
                ps_b = ps_f2.tile([128, 512], F32, tag="f2")
                for f in range(FT):
                    lhsT = intert[f][:, 128 * s:128 * (s + 1)]
                    nc.tensor.matmul(ps_a, lhsT, wo2r[f][:, 0:512],
                                     start=(f == 0), stop=(f == FT - 1))
                    nc.tensor.matmul(ps_b, lhsT, wo2r[f][:, 512:1024],
                                     start=(f == 0), stop=(f == FT - 1))
                z2 = p_z2.tile([128, H], BF16, tag="z2")
                for hh, pso in ((0, ps_a), (1, ps_b)):
                    nc.vector.tensor_tensor(
                        out=z2[:, 512 * hh:512 * (hh + 1)], in0=pso,
                        in1=aotbo[s][:, 512 * hh:512 * (hh + 1)], op=ALU.add)
                s1 = p_st2.tile([128, 1], F32, tag="s1")
                nc.vector.tensor_reduce(out=s1, in_=z2, axis=mybir.AxisListType.X,
                                        op=ALU.add)
                sq2 = p_sq2.tile([128, H], BF16, tag="sq2")
                s2 = p_st2.tile([128, 1], F32, tag="s2")
                nc.scalar.activation(out=sq2, in_=z2, func=AF.Square,
                                     accum_out=s2)
                mu = p_st2.tile([128, 1], F32, tag="mu")
                nc.vector.tensor_scalar(out=mu, in0=s1, scalar1=1.0 / H,
                                        scalar2=None, op0=ALU.mult)
                ex2b = p_st2.tile([128, 1], F32, tag="ex2b")
                nc.vector.tensor_scalar(out=ex2b, in0=s2, scalar1=1.0 / H,
                                        scalar2=None, op0=ALU.mult)
                mu2b = p_st2.tile([128, 1], F32, tag="mu2b")
                nc.vector.tensor_tensor(out=mu2b, in0=mu, in1=mu, op=ALU.mult)
                varb = p_st2.tile([128, 1], F32, tag="varb")
                nc.vector.tensor_tensor(out=varb, in0=ex2b, in1=mu2b,
                                        op=ALU.subtract)
                sdb = p_st2.tile([128, 1], F32, tag="sdb")
                nc.scalar.activation(out=sdb, in_=varb, func=AF.Sqrt,
                                     bias=t_eps128, scale=1.0)
                rstdb = p_st2.tile([128, 1], F32, tag="rstdb")
                nc.vector.reciprocal(out=rstdb, in_=sdb)
                nmu = p_st2.tile([128, 1], F32, tag="nmu")
                nc.vector.tensor_tensor(out=nmu, in0=mu, in1=rstdb, op=ALU.mult)
                nb = p_st2.tile([128, 1], F32, tag="nb")
                nc.vector.tensor_scalar(out=nb, in0=nmu, scalar1=-1.0,
                                        scalar2=None, op0=ALU.mult)
                y1 = p_y.tile([128, H], BF16, tag="y1")
                nc.scalar.activation(out=y1, in_=z2, func=AF.Identity,
                                     bias=nb, scale=rstdb)
                y2 = p_y.tile([128, H], BF16, tag="y2")
                nc.vector.tensor_tensor(out=y2, in0=y1, in1=t_g2r, op=ALU.mult)
                y3 = p_y.tile([128, H], BF16, tag="y3")
                nc.vector.tensor_tensor(out=y3, in0=y2, in1=t_b2r, op=ALU.add)
                nc.sync.dma_start(out=out_d[128 * s:128 * (s + 1), :], in_=y3)
        es_int.close()

    nc.compile()
    return nc


def _get_nc():
    if "nc" not in _CACHE:
        _CACHE["nc"] = _build()
    return _CACHE["nc"]


def _perpart(v):
    # [n*128] -> [128, n] with vT[p, t] = v[t*128 + p]
    v = np.asarray(v, np.float32)
    return np.ascontiguousarray(v.reshape(-1, 128).T)


def _pack_pairs(w, scale, dtype):
    # w [K, N] -> [K/2, 2N] fp8: row (128p + k) holds [i in {0,1}, n] with
    # value w[(2p+i)*128 + k, n] * scale
    w = np.asarray(w, np.float32) * scale
    K, N = w.shape
    t = w.reshape(K // 256, 2, 128, N).transpose(0, 2, 1, 3)
    return np.ascontiguousarray(t.reshape(K // 2, 2 * N)).astype(dtype)


def _ones8():
    # [128, (g, i, c)]: g=0: cols 0:64 ones; g=1: cols 64:128 ones
    a = np.zeros((128, 2, 2, 128), np.float32)
    a[:, 0, :, 0:64] = 1.0
    a[:, 1, :, 64:128] = 1.0
    return a.reshape(128, 512).astype(ml_dtypes.float8_e4m3)


def _shared_inputs(inp):
    f8 = ml_dtypes.float8_e4m3
    bf = ml_dtypes.bfloat16
    f = np.float32
    biast = np.concatenate(
        [_perpart(inp["bq"]), _perpart(inp["bk"]), _perpart(inp["bo"]),
         _perpart(inp["ln1_g"]), _perpart(inp["ln1_b"]), _perpart(inp["bi"])],
        axis=1).astype(f)
    return {
        "wq8": _pack_pairs(inp["wq"], WS, f8),
        "wk8": _pack_pairs(inp["wk"], WS, f8),
        "wv8": _pack_pairs(inp["wv"], WS, f8),
        "wo8": _pack_pairs(inp["wo"], WS, f8),
        "wi": np.ascontiguousarray(np.asarray(inp["wi"], f)).astype(bf),
        "wo2": np.ascontiguousarray(np.asarray(inp["wo2"], f)).astype(bf),
        "biast": biast,
        "bv16": (np.asarray(inp["bv"], f) * WS).reshape(1, H).astype(bf),
        "bo2r": np.asarray(inp["bo2"], f).reshape(1, H).astype(bf),
        "g2r": np.asarray(inp["ln2_g"], f).reshape(1, H).astype(bf),
        "b2r": np.asarray(inp["ln2_b"], f).reshape(1, H).astype(bf),
        "ones8": _ones8(),
        "onec": np.ones((128, 1), f).astype(bf),
        "oner": np.ones((1, 128), f),
    }


def _core_inputs(shared, hs, am, b):
    f8 = ml_dtypes.float8_e4m3
    bf = ml_dtypes.bfloat16
    xT = np.ascontiguousarray(hs[b].T)  # [H, S] fp32
    return dict(
        shared,
        xt8=_pack_pairs(xT, 1.0, f8),
        xtb=xT.astype(bf),
        maskb=(_perpart(am[b]) + EXP_SHIFT).astype(np.float32),
    )


def kernel(hidden_states, attention_mask, wq, bq, wk, bk, wv, bv,
           wo, bo, ln1_g, ln1_b, wi, bi, wo2, bo2, ln2_g, ln2_b):
    nc = _get_nc()
    f = np.float32
    shared = _shared_inputs({
        "wq": wq, "wk": wk, "wv": wv, "wo": wo, "wi": wi, "wo2": wo2,
        "bq": bq, "bk": bk, "bv": bv, "bo": bo, "bi": bi, "bo2": bo2,
        "ln1_g": ln1_g, "ln1_b": ln1_b, "ln2_g": ln2_g, "ln2_b": ln2_b,
    })
    hs = np.ascontiguousarray(hidden_states, f)
    am = np.ascontiguousarray(attention_mask, f).reshape(B, S)
    in_maps = [_core_inputs(shared, hs, am, b) for b in range(B)]
    res = bass_utils.run_bass_kernel_spmd(nc, in_maps, core_ids=list(range(B)),
                                          trace=False)
    return np.stack([res.results[b]["out"].astype(f) for b in range(B)])
